# revision 53
# baseline (speedup 1.0000x reference)
"""Multi-head attention (B=4, S=2048, D=1024, H=16) on 8 Trainium2 cores.

Sharding: data-parallel over batch (4) x tensor-parallel over heads (2).
Core c handles batch c//2 and heads (c%2)*8 .. +8.  Each core computes a
partial output (its heads' contribution through the O-projection); the host
sums the two partials per batch and adds the output bias.

Single fused pipeline: the softmax exp stream on ScalarE (33.5M elem/core,
~1 elem/cycle/lane) is the hard floor, so the attention loop starts after a
minimal prelude (kT[0]/qT[0] first chunks + v head chunk 0) and the rest of
the q/k/v projections plus the O-projection are drained into TensorE's spare
capacity through a spill queue interleaved with the attention iterations.
Scores stay transposed (S^T[k,q]); the softmax denominator rides the AV
matmul as a 65th V column of ones; the denominator reciprocal runs on a
[128,8]-reshaped view (cheap) instead of a single-partition [1,512] vector.
"""

import numpy as np
from collections import deque
from contextlib import ExitStack

import ml_dtypes
import concourse.bass as bass
import concourse.tile as tile
from concourse import bacc, mybir
from concourse.bass import ts
from concourse.bass_utils import run_bass_kernel_spmd

P = 128
S = 2048          # sequence length
D = 1024          # model dim
DOUT = 512        # per-core projection width (8 heads x 64)
DK = 64           # head dim
B = 4
N_CORES = 8
F32 = mybir.dt.float32
BF16 = mybir.dt.bfloat16
FP = mybir.ActivationFunctionType

NKC = D // P      # 8 contraction chunks over model dim
NM = DOUT // P    # 4 dout chunks (also head pairs)
NQ = S // 512     # 4 query chunks of 512
NK16 = S // P     # 16 key chunks of 128

_cached_nc = None


def _emit(ctx: ExitStack, tc: "tile.TileContext", io: dict):
    nc = tc.nc

    qt_r = io["qt"].ap().rearrange("(c p) s -> p c s", p=P)      # [128, 8, 2048]
    kt_r = io["kt"].ap().rearrange("(c p) s -> p c s", p=P)
    vt_r = io["vt"].ap().rearrange("(c p) s -> p c s", p=P)
    wqt_r = io["wqt"].ap().rearrange("(c p) m -> p c m", p=P)    # [128, 8, 512]
    wkt_r = io["wkt"].ap().rearrange("(c p) m -> p c m", p=P)
    wvt_r = io["wvt"].ap().rearrange("(c p) m -> p c m", p=P)
    wot_r = io["wot"].ap().rearrange("(c p) n -> p c n", p=P)    # [128, 4, 1024]
    # host supplies bq/bk pre-shuffled to partition-major [128, 4] each so the
    # DMA is one clean burst (the naive [512]->(c p) pattern is 4-byte strided)
    bqk_r = io["bqk"].ap().rearrange("(p c) -> p c", p=P)        # [128, 8]
    bv_ap = io["bv"].ap()                                        # [512]
    out_r = io["out"].ap().rearrange("(sc p) n -> p sc n", p=P)  # [128, 16, 1024]

    persist = ctx.enter_context(tc.tile_pool(name="persist", bufs=1))
    kinp = ctx.enter_context(tc.tile_pool(name="kinp", bufs=4))
    qinp = ctx.enter_context(tc.tile_pool(name="qinp", bufs=3))
    vinp = ctx.enter_context(tc.tile_pool(name="vinp", bufs=2))
    etp = ctx.enter_context(tc.tile_pool(name="etp", bufs=9))
    avsb = ctx.enter_context(tc.tile_pool(name="avsb", bufs=4))
    recipp = ctx.enter_context(tc.tile_pool(name="recipp", bufs=1))
    stagp = ctx.enter_context(tc.tile_pool(name="stagp", bufs=1))
    outp = ctx.enter_context(tc.tile_pool(name="outp", bufs=2))
    dramp = ctx.enter_context(tc.tile_pool(name="dramp", bufs=3, space="DRAM"))

    ps_st = ctx.enter_context(tc.tile_pool(name="ps_st", bufs=2, space="PSUM"))
    ps_av = ctx.enter_context(tc.tile_pool(name="ps_av", bufs=2, space="PSUM"))
    ps_pj = ctx.enter_context(tc.tile_pool(name="ps_pj", bufs=2, space="PSUM"))

    # ---- prelude-critical DMAs, bandwidth-staged ----------------------------
    # Wave 1 (parallel queues): kin si0 (sync), wk (gpsimd), qin si0 (scalar).
    # Wave 2: vin g0 (sync), wv (gpsimd), wq (scalar).  Everything else (kin
    # si1-3, wo, qin si1-3, vin g1-3) is deferred so it doesn't steal HBM
    # bandwidth from the pipeline-critical prelude bytes.
    kin = {}

    def load_kin(si, eng=None):
        t = kinp.tile([P, NKC, 512], BF16, tag="kin", name=f"kin{si}")
        (eng or nc.sync).dma_start(out=t, in_=kt_r[:, :, ts(si, 512)])
        kin[si] = t

    # kin si0 in two key-slabs (256 keys each) so the first kT matmuls start
    # after only 512KB of input bytes
    kin0 = kinp.tile([P, NKC, 512], BF16, tag="kin", name="kin0")
    nc.sync.dma_start(out=kin0[:, :, 0:256], in_=kt_r[:, :, 0:256])
    nc.sync.dma_start(out=kin0[:, :, 256:512], in_=kt_r[:, :, 256:512])
    kin[0] = kin0
    # weight loads split by m-columns: the whole pc=0 pipeline (kT[0]/qT[0])
    # only touches the m=0 column slice (256KB); the rest streams later
    wk_sb = persist.tile([P, NKC, DOUT], BF16, tag="wk")
    nc.gpsimd.dma_start(out=wk_sb[:, :, 0:P], in_=wkt_r[:, :, 0:P])
    bqk_sb = persist.tile([P, 2 * NM], F32, tag="bqk")
    nc.gpsimd.dma_start(out=bqk_sb, in_=bqk_r)
    wq_sb = persist.tile([P, NKC, DOUT], BF16, tag="wq")
    nc.gpsimd.dma_start(out=wq_sb[:, :, 0:P], in_=wqt_r[:, :, 0:P])
    wv_sb = persist.tile([P, NKC, DOUT], BF16, tag="wv")
    nc.gpsimd.dma_start(out=wv_sb, in_=wvt_r)
    bv_rep = persist.tile([P, DOUT], F32, tag="bvrep")
    bv_bcast = bass.AP(
        tensor=bv_ap.tensor, offset=bv_ap.offset, ap=[[0, P]] + list(bv_ap.ap)
    )
    nc.gpsimd.dma_start(out=bv_rep, in_=bv_bcast)
    nc.gpsimd.dma_start(out=wk_sb[:, :, P:DOUT], in_=wkt_r[:, :, P:DOUT])
    nc.gpsimd.dma_start(out=wq_sb[:, :, P:DOUT], in_=wqt_r[:, :, P:DOUT])

    # ---- persistent activations (bf16) --------------------------------------
    # qT / kT: [dout, s] as 4 chunk-tiles of [128, 2048] (chunk = head pair)
    qT = [persist.tile([P, S], BF16, tag=f"qT{m}", name=f"qT{m}") for m in range(NM)]
    kT = [persist.tile([P, S], BF16, tag=f"kT{m}", name=f"kT{m}") for m in range(NM)]
    # v: [s, head, dk+1] tiles; col 64 of each head block holds ones so the
    # AV matmul's 65th output row accumulates the softmax denominator
    v_sb = [
        persist.tile([P, 8, 65], BF16, tag=f"v{i}", name=f"v{i}") for i in range(NK16)
    ]
    for i in range(NK16):
        nc.vector.memset(v_sb[i][:, :, 64:65], 1.0)
    # attn_outT: [dout, s] as 4 chunk-tiles (rows 0-63 even head, 64-127 odd)
    aoT = [persist.tile([P, S], BF16, tag=f"aoT{m}", name=f"aoT{m}") for m in range(NM)]

    # wo is only needed by the O-projection (pc=3) — DMA'd mid-attention
    wo_sb = persist.tile([P, NM, D], BF16, tag="wo")

    qin = {}

    def load_qin(si, eng=None):
        if si not in qin:
            qin[si] = qinp.tile([P, NKC, 512], BF16, tag="qin", name=f"qin{si}")
        (eng or nc.sync).dma_start(out=qin[si], in_=qt_r[:, :, ts(si, 512)])

    vin = {}

    def load_vin(g, eng=None):
        t = vinp.tile([P, NKC, 512], BF16, tag="vin", name=f"vin{g}")
        (eng or nc.sync).dma_start(out=t, in_=vt_r[:, :, ts(g, 512)])
        vin[g] = t

    # ---- projection chunk emitters ------------------------------------------
    def proj_chunk(w_sb, xin, m, dst, si, bias_base):
        """dst[m][:, si*512:(si+1)*512] = (W[m] @ x[si]) + b[m]  (8 MMs)."""
        ps = ps_pj.tile([P, 512], F32, tag="pj", name="pspj")
        for kc in range(NKC):
            nc.tensor.matmul(
                ps,
                lhsT=w_sb[:, kc, ts(m, P)],
                rhs=xin[:, kc, :],
                start=(kc == 0),
                stop=(kc == NKC - 1),
                skip_group_check=True,
            )
        c = bias_base + m
        nc.vector.tensor_add(
            out=dst[m][:, ts(si, 512)],
            in0=ps,
            in1=bqk_sb[:, c : c + 1].to_broadcast([P, 512]),
        )

    def vproj_block(g, j):
        """v_sb[4g+j][:, :, 0:64] = (v[si16] @ Wv^T) + bv  (8 MMs)."""
        si16 = g * 4 + j
        ps = ps_pj.tile([P, 512], F32, tag="pj", name="pspjv")
        for kc in range(NKC):
            nc.tensor.matmul(
                ps,
                lhsT=vin[g][:, kc, ts(j, P)],
                rhs=wv_sb[:, kc, :],
                start=(kc == 0),
                stop=(kc == NKC - 1),
                skip_group_check=True,
            )
        nc.vector.tensor_add(
            out=v_sb[si16][:, :, 0:64],
            in0=ps.rearrange("p (h d) -> p h d", h=8),
            in1=bv_rep.rearrange("p (h d) -> p h d", h=8),
        )

    # ---- prelude: unblock the attention pipeline ASAP -----------------------
    load_qin(0, eng=nc.scalar)
    load_vin(0, eng=nc.sync)
    # kT[0] si0 in two 256-key slabs so ST(ki=0,1) unblocks after slab 0
    for sl in range(2):
        ps = ps_pj.tile([P, 512], F32, tag="pj", name="pspj")
        for kc in range(NKC):
            nc.tensor.matmul(
                ps[:, 0:256],
                lhsT=wk_sb[:, kc, 0:P],
                rhs=kin0[:, kc, sl * 256 : sl * 256 + 256],
                start=(kc == 0),
                stop=(kc == NKC - 1),
                skip_group_check=True,
            )
        nc.vector.tensor_add(
            out=kT[0][:, sl * 256 : sl * 256 + 256],
            in0=ps[:, 0:256],
            in1=bqk_sb[:, NM : NM + 1].to_broadcast([P, 256]),
        )
    proj_chunk(wq_sb, qin[0], 0, qT, 0, 0)
    for j in range(4):
        vproj_block(0, j)

    # ---- spill queue: remaining projection work, deadline-paced -------------
    # Each entry is (deadline_iter, emit_fn); iter = pc*64 + qi*16 + ki of the
    # attention iteration that first consumes the chunk (with slack baked in).
    items = []

    def sp_k(m, si, dl, load=False):
        def emit(m=m, si=si, load=load):
            if load:
                load_kin(si)
            proj_chunk(wk_sb, kin[si], m, kT, si, NM)

        items.append((dl, emit))

    def sp_q(m, si, dl, load=False):
        def emit(m=m, si=si, load=load):
            if load:
                load_qin(si)
            proj_chunk(wq_sb, qin[si], m, qT, si, 0)

        items.append((dl, emit))

    def sp_v(g, j, dl):
        def emit(g=g, j=j):
            if j == 0:
                load_vin(g)
            vproj_block(g, j)

        items.append((dl, emit))

    for si in range(1, NQ):
        sp_k(0, si, 4 * si, load=True)
    for g in range(1, NQ):
        for j in range(4):
            sp_v(g, j, 4 * g + j + 4)
    sp_q(0, 1, 16, load=True)
    sp_q(0, 2, 32, load=True)
    # qT[1..3]@si0 must be emitted before the qin si3 load reuses its slot
    sp_q(1, 0, 40)
    sp_q(2, 0, 41)
    sp_q(3, 0, 42)
    sp_q(0, 3, 48, load=True)
    for m in range(1, NM):
        for si in range(NQ):
            sp_k(m, si, 64 * m + 4 * si)
    for m in range(1, NM):
        for si in range(1, NQ):
            sp_q(m, si, 64 * m + 16 * si)
    items.sort(key=lambda e: e[0])
    spill = deque(items)

    def drain(it):
        """Deadline-driven: emit chunks ~30 iters before first use (the spill
        matmuls only get TensorE's spare ~0.35us/iter, so a chunk needs many
        iterations of headroom); spread the surplus so filler work survives
        into the late ACT-bound phase."""
        hit = False
        look = 30 if it < 96 else 45
        while spill and spill[0][0] <= it + look:
            spill.popleft()[1]()
            hit = True
        if (
            not hit
            and spill
            and it % 3 == 2
            and len(spill) > (4 * 64 - it) // 16
        ):
            spill.popleft()[1]()

    def drain_rest():
        while spill:
            spill.popleft()[1]()

    # ---- O-projection half-chunk (4 MMs; small enough not to displace exp) --
    def oproj_half(si16, n2):
        ps = ps_pj.tile([P, 512], F32, tag="pj", name="pso")
        for c in range(NM):
            nc.tensor.matmul(
                ps,
                lhsT=aoT[c][:, ts(si16, P)],
                rhs=wo_sb[:, c, ts(n2, 512)],
                start=(c == 0),
                stop=(c == NM - 1),
                skip_group_check=True,
            )
        osb = outp.tile([P, 512], F32, tag="osb")
        nc.vector.tensor_copy(out=osb, in_=ps)
        nc.gpsimd.dma_start(out=out_r[:, si16, ts(n2, 512)], in_=osb)

    def oproj_chunk(si16):
        oproj_half(si16, 0)
        oproj_half(si16, 1)

    # ---- attention (scores kept transposed: tiles are S^T[k, q]) ------------
    # head pair pc: even head = partitions/cols 0-63, odd head = 64-127
    for pc in range(NM):
        hh = 2 * pc
        if pc == 2:
            # load the O-projection weights well before pc=3 needs them
            # (sync queue: FIFO-staged behind the earlier input loads)
            nc.sync.dma_start(out=wo_sb, in_=wot_r)
        for qi in range(NQ):
            av_e = ps_av.tile([P, 512], F32, tag="av", name="av_e")  # rows 0-64
            av_o = ps_av.tile([P, 512], F32, tag="av", name="av_o")
            for ki in range(NK16):
                st = ps_st.tile([P, 1024], F32, tag="st", name="st")
                nc.tensor.matmul(
                    st[:, 0:512],
                    lhsT=kT[pc][0:64, ts(ki, P)],
                    rhs=qT[pc][0:64, ts(qi, 512)],
                    start=True,
                    stop=True,
                )
                nc.tensor.matmul(
                    st[:, 512:1024],
                    lhsT=kT[pc][64:128, ts(ki, P)],
                    rhs=qT[pc][64:128, ts(qi, 512)],
                    start=True,
                    stop=True,
                    skip_group_check=True,
                )
                et = etp.tile([P, 1024], BF16, tag="et", name="et")
                nc.scalar.activation(out=et, in_=st, func=FP.Exp, scale=0.125)
                first = ki == 0
                last = ki == NK16 - 1
                nc.tensor.matmul(
                    av_e[0:65],
                    lhsT=v_sb[ki][:, hh, :],
                    rhs=et[:, 0:512],
                    start=first,
                    stop=last,
                    skip_group_check=True,
                )
                nc.tensor.matmul(
                    av_o[0:65],
                    lhsT=v_sb[ki][:, hh + 1, :],
                    rhs=et[:, 512:1024],
                    start=first,
                    stop=last,
                    skip_group_check=True,
                )
                drain(64 * pc + 16 * qi + ki)
                # O-projection for the previous qi's s-range: six half-chunks
                # spread through the ki loop; the last chunk is reserved as a
                # filler bridging this qi's normalization latency
                if pc == NM - 1 and qi > 0 and ki % 2 == 1 and ki < 12:
                    oproj_half(4 * (qi - 1) + ki // 4, (ki // 2) % 2)
            # free the PSUM banks fast, then normalize from SBUF
            ae = avsb.tile([P, 512], F32, tag="ae", name="ae")
            ao = avsb.tile([P, 512], F32, tag="ae", name="ao")
            nc.vector.tensor_copy(out=ae[0:65], in_=av_e[0:65])
            nc.vector.tensor_copy(out=ao[0:65], in_=av_o[0:65])
            # denominators: bounce [2,512] to DRAM, reshape to [128,8] so the
            # iterative-divide reciprocal runs across partitions, not 1 lane
            scr = dramp.tile([2, 512], F32, tag="scr", name="scr")
            nc.sync.dma_start(out=scr[0:1, :], in_=ae[64:65, :])
            nc.sync.dma_start(out=scr[1:2, :], in_=ao[64:65, :])
            den = recipp.tile([P, 8], F32, tag="den", name="den")
            scr_flat = scr[:, :].rearrange("a b -> (a b)").rearrange(
                "(p f) -> p f", p=P
            )
            nc.sync.dma_start(out=den, in_=scr_flat)
            denr = recipp.tile([P, 8], F32, tag="denr", name="denr")
            nc.vector.reciprocal(out=denr, in_=den)
            scr2 = dramp.tile([P, 8], F32, tag="scr2", name="scr2")
            nc.sync.dma_start(out=scr2, in_=denr)
            rep = recipp.tile([P, 1024], F32, tag="rep", name="rep")
            s2f = scr2[:, :].rearrange("p f -> (p f)")
            nc.sync.dma_start(
                out=rep[0:64, :],
                in_=bass.AP(
                    tensor=s2f.tensor, offset=s2f.offset, ap=[[0, 64]] + list(s2f.ap)
                ),
            )
            last_block = pc == NM - 1 and qi == NQ - 1
            if not last_block:
                nc.vector.tensor_mul(
                    out=aoT[pc][0:64, ts(qi, 512)], in0=ae[0:64], in1=rep[0:64, 0:512]
                )
                stag = stagp.tile([P, 512], BF16, tag="stag", name="stag")
                nc.vector.tensor_mul(
                    out=stag[0:64, :], in0=ao[0:64], in1=rep[0:64, 512:1024]
                )
                nc.sync.dma_start(
                    out=aoT[pc][64:128, ts(qi, 512)], in_=stag[0:64, :]
                )
                if pc == NM - 1 and qi > 0:
                    # reserved filler: keep TensorE warm behind the norm chain
                    oproj_chunk(4 * (qi - 1) + 3)
            else:
                # final block: normalize in 128-col pieces, each immediately
                # feeding its O-projection chunk so the tail pipelines;
                # qi=2's last chunk bridges the reciprocal chain latency
                oproj_chunk(4 * (qi - 1) + 3)
                for piece in range(4):
                    cs = qi * 512 + piece * 128
                    ps_lo = slice(piece * 128, piece * 128 + 128)
                    ps_hi = slice(512 + piece * 128, 512 + piece * 128 + 128)
                    nc.vector.tensor_mul(
                        out=aoT[pc][0:64, cs : cs + 128],
                        in0=ae[0:64, piece * 128 : piece * 128 + 128],
                        in1=rep[0:64, ps_lo],
                    )
                    stag = stagp.tile([P, 128], BF16, tag="stagf", name="stagf")
                    nc.vector.tensor_mul(
                        out=stag[0:64, :],
                        in0=ao[0:64, piece * 128 : piece * 128 + 128],
                        in1=rep[0:64, ps_hi],
                    )
                    nc.sync.dma_start(
                        out=aoT[pc][64:128, cs : cs + 128], in_=stag[0:64, :]
                    )
                    oproj_chunk(4 * qi + piece)

    drain_rest()


def _build():
    global _cached_nc
    if _cached_nc is not None:
        return _cached_nc
    nc = bacc.Bacc("TRN2", target_bir_lowering=False, debug=False)
    io = {
        "qt": nc.dram_tensor("qt", [D, S], BF16, kind="ExternalInput"),
        "kt": nc.dram_tensor("kt", [D, S], BF16, kind="ExternalInput"),
        "vt": nc.dram_tensor("vt", [D, S], BF16, kind="ExternalInput"),
        "wqt": nc.dram_tensor("wqt", [D, DOUT], BF16, kind="ExternalInput"),
        "wkt": nc.dram_tensor("wkt", [D, DOUT], BF16, kind="ExternalInput"),
        "wvt": nc.dram_tensor("wvt", [D, DOUT], BF16, kind="ExternalInput"),
        "wot": nc.dram_tensor("wot", [DOUT, D], BF16, kind="ExternalInput"),
        "bqk": nc.dram_tensor("bqk", [2 * DOUT], F32, kind="ExternalInput"),
        "bv": nc.dram_tensor("bv", [DOUT], F32, kind="ExternalInput"),
        "out": nc.dram_tensor("out", [S, D], F32, kind="ExternalOutput"),
    }
    with tile.TileContext(nc) as tc:
        with ExitStack() as ctx:
            _emit(ctx, tc, io)
    nc.compile()
    _cached_nc = nc
    return nc


def make_in_maps(Q, K, V, Wq, bq, Wk, bk, Wv, bv, Wo):
    bf = lambda a: np.ascontiguousarray(np.asarray(a, np.float32)).astype(
        ml_dtypes.bfloat16
    )
    f = lambda a: np.ascontiguousarray(a, dtype=np.float32)
    in_maps = []
    for c in range(N_CORES):
        b = c // 2
        lo = (c % 2) * DOUT
        sl = slice(lo, lo + DOUT)
        in_maps.append(
            {
                "qt": bf(np.asarray(Q, np.float32)[b].T),
                "kt": bf(np.asarray(K, np.float32)[b].T),
                "vt": bf(np.asarray(V, np.float32)[b].T),
                "wqt": bf(np.asarray(Wq, np.float32)[sl, :].T),
                "wkt": bf(np.asarray(Wk, np.float32)[sl, :].T),
                "wvt": bf(np.asarray(Wv, np.float32)[sl, :].T),
                "wot": bf(np.asarray(Wo, np.float32)[:, sl].T),
                # partition-major shuffle: bqk[p, c] = bias[c*128 + p]
                "bqk": f(
                    np.concatenate(
                        [
                            np.asarray(bq[sl]).reshape(NM, P).T,
                            np.asarray(bk[sl]).reshape(NM, P).T,
                        ],
                        axis=1,
                    ).reshape(-1)
                ),
                "bv": f(bv[sl]),
            }
        )
    return in_maps


def gather_output(results, bo):
    out = np.empty((B, S, D), dtype=np.float32)
    bo = np.asarray(bo, dtype=np.float32)
    for b in range(B):
        out[b] = results[2 * b]["out"] + results[2 * b + 1]["out"] + bo
    return out


def _numpy_fallback(Q, K, V, mask, Wq, bq, Wk, bk, Wv, bv, Wo, bo):
    """Exact reference math in numpy (only used if mask isn't all-ones)."""
    H, dk = 16, 64
    out = np.empty((B, S, D), dtype=np.float32)
    for b in range(B):
        q = (Q[b] @ Wq.T + bq).reshape(S, H, dk).transpose(1, 0, 2)
        k = (K[b] @ Wk.T + bk).reshape(S, H, dk).transpose(1, 0, 2)
        v = (V[b] @ Wv.T + bv).reshape(S, H, dk).transpose(1, 0, 2)
        o = np.empty((H, S, dk), dtype=np.float32)
        for h in range(H):
            s = (q[h] @ k[h].T) / np.sqrt(np.float32(dk))
            s = np.where(mask[b] == 0, np.float32(-1.0e9), s)
            s = s - s.max(axis=-1, keepdims=True)
            e = np.exp(s)
            a = e / e.sum(axis=-1, keepdims=True)
            o[h] = a @ v[h]
        out[b] = o.transpose(1, 0, 2).reshape(S, H * dk) @ Wo.T + bo
    return out


def kernel(Q, K, V, mask, Wq, bq, Wk, bk, Wv, bv, Wo, bo):
    Q = np.asarray(Q, dtype=np.float32)
    K = np.asarray(K, dtype=np.float32)
    V = np.asarray(V, dtype=np.float32)
    Wq = np.asarray(Wq, dtype=np.float32)
    Wk = np.asarray(Wk, dtype=np.float32)
    Wv = np.asarray(Wv, dtype=np.float32)
    Wo = np.asarray(Wo, dtype=np.float32)
    bq = np.asarray(bq, dtype=np.float32)
    bk = np.asarray(bk, dtype=np.float32)
    bv = np.asarray(bv, dtype=np.float32)
    bo = np.asarray(bo, dtype=np.float32)
    mask_np = np.asarray(mask)

    if not np.all(mask_np != 0):
        return _numpy_fallback(Q, K, V, mask_np, Wq, bq, Wk, bk, Wv, bv, Wo, bo)

    nc = _build()
    in_maps = make_in_maps(Q, K, V, Wq, bq, Wk, bk, Wv, bv, Wo)
    res = run_bass_kernel_spmd(nc, in_maps, list(range(N_CORES))).results
    return gather_output(res, bo)


# revision 54
# speedup vs baseline: 1.0118x; 1.0118x over previous
"""Multi-head attention (B=4, S=2048, D=1024, H=16) on 8 Trainium2 cores.

Sharding: data-parallel over batch (4) x tensor-parallel over heads (2).
Core c handles batch c//2 and heads (c%2)*8 .. +8.  Each core computes a
partial output (its heads' contribution through the O-projection); the host
sums the two partials per batch and adds the output bias.

Single fused pipeline: the softmax exp stream on ScalarE (33.5M elem/core,
~1 elem/cycle/lane) is the hard floor, so the attention loop starts after a
minimal prelude (kT[0]/qT[0] first chunks + v head chunk 0) and the rest of
the q/k/v projections plus the O-projection are drained into TensorE's spare
capacity through a spill queue interleaved with the attention iterations.
Scores stay transposed (S^T[k,q]); the softmax denominator rides the AV
matmul as a 65th V column of ones; the denominator reciprocal runs on a
[128,8]-reshaped view (cheap) instead of a single-partition [1,512] vector.
"""

import numpy as np
from collections import deque
from contextlib import ExitStack

import ml_dtypes
import concourse.bass as bass
import concourse.tile as tile
from concourse import bacc, mybir
from concourse.bass import ts
from concourse.bass_utils import run_bass_kernel_spmd

P = 128
S = 2048          # sequence length
D = 1024          # model dim
DOUT = 512        # per-core projection width (8 heads x 64)
DK = 64           # head dim
B = 4
N_CORES = 8
F32 = mybir.dt.float32
BF16 = mybir.dt.bfloat16
FP = mybir.ActivationFunctionType

NKC = D // P      # 8 contraction chunks over model dim
NM = DOUT // P    # 4 dout chunks (also head pairs)
NQ = S // 512     # 4 query chunks of 512
NK16 = S // P     # 16 key chunks of 128

_cached_nc = None


def _emit(ctx: ExitStack, tc: "tile.TileContext", io: dict):
    nc = tc.nc

    qt_r = io["qt"].ap().rearrange("(c p) s -> p c s", p=P)      # [128, 8, 2048]
    kt_r = io["kt"].ap().rearrange("(c p) s -> p c s", p=P)
    vt_r = io["vt"].ap().rearrange("(c p) s -> p c s", p=P)
    wqt_r = io["wqt"].ap().rearrange("(c p) m -> p c m", p=P)    # [128, 8, 512]
    wkt_r = io["wkt"].ap().rearrange("(c p) m -> p c m", p=P)
    wvt_r = io["wvt"].ap().rearrange("(c p) m -> p c m", p=P)
    wot_r = io["wot"].ap().rearrange("(c p) n -> p c n", p=P)    # [128, 4, 1024]
    # host supplies bq/bk pre-shuffled to partition-major [128, 4] each so the
    # DMA is one clean burst (the naive [512]->(c p) pattern is 4-byte strided)
    bqk_r = io["bqk"].ap().rearrange("(p c) -> p c", p=P)        # [128, 8]
    bv_ap = io["bv"].ap()                                        # [512]
    out_r = io["out"].ap().rearrange("(sc p) n -> p sc n", p=P)  # [128, 16, 1024]

    persist = ctx.enter_context(tc.tile_pool(name="persist", bufs=1))
    kinp = ctx.enter_context(tc.tile_pool(name="kinp", bufs=4))
    qinp = ctx.enter_context(tc.tile_pool(name="qinp", bufs=3))
    vinp = ctx.enter_context(tc.tile_pool(name="vinp", bufs=2))
    etp = ctx.enter_context(tc.tile_pool(name="etp", bufs=7))
    avsb = ctx.enter_context(tc.tile_pool(name="avsb", bufs=3))
    recipp = ctx.enter_context(tc.tile_pool(name="recipp", bufs=2))
    stagp = ctx.enter_context(tc.tile_pool(name="stagp", bufs=2))
    outp = ctx.enter_context(tc.tile_pool(name="outp", bufs=3))
    dramp = ctx.enter_context(tc.tile_pool(name="dramp", bufs=3, space="DRAM"))

    ps_st = ctx.enter_context(tc.tile_pool(name="ps_st", bufs=2, space="PSUM"))
    ps_av = ctx.enter_context(tc.tile_pool(name="ps_av", bufs=2, space="PSUM"))
    ps_pj = ctx.enter_context(tc.tile_pool(name="ps_pj", bufs=2, space="PSUM"))

    # ---- prelude-critical DMAs, bandwidth-staged ----------------------------
    # Wave 1 (parallel queues): kin si0 (sync), wk (gpsimd), qin si0 (scalar).
    # Wave 2: vin g0 (sync), wv (gpsimd), wq (scalar).  Everything else (kin
    # si1-3, wo, qin si1-3, vin g1-3) is deferred so it doesn't steal HBM
    # bandwidth from the pipeline-critical prelude bytes.
    kin = {}

    def load_kin(si, eng=None):
        t = kinp.tile([P, NKC, 512], BF16, tag="kin", name=f"kin{si}")
        (eng or nc.sync).dma_start(out=t, in_=kt_r[:, :, ts(si, 512)])
        kin[si] = t

    # kin si0 in two key-slabs (256 keys each) so the first kT matmuls start
    # after only 512KB of input bytes
    kin0 = kinp.tile([P, NKC, 512], BF16, tag="kin", name="kin0")
    nc.sync.dma_start(out=kin0[:, :, 0:256], in_=kt_r[:, :, 0:256])
    nc.sync.dma_start(out=kin0[:, :, 256:512], in_=kt_r[:, :, 256:512])
    kin[0] = kin0
    # weight loads split by m-columns: the whole pc=0 pipeline (kT[0]/qT[0])
    # only touches the m=0 column slice (256KB); the rest streams later
    wk_sb = persist.tile([P, NKC, DOUT], BF16, tag="wk")
    nc.gpsimd.dma_start(out=wk_sb[:, :, 0:P], in_=wkt_r[:, :, 0:P])
    bqk_sb = persist.tile([P, 2 * NM], F32, tag="bqk")
    nc.gpsimd.dma_start(out=bqk_sb, in_=bqk_r)
    wq_sb = persist.tile([P, NKC, DOUT], BF16, tag="wq")
    nc.gpsimd.dma_start(out=wq_sb[:, :, 0:P], in_=wqt_r[:, :, 0:P])
    wv_sb = persist.tile([P, NKC, DOUT], BF16, tag="wv")
    nc.gpsimd.dma_start(out=wv_sb, in_=wvt_r)
    bv_rep = persist.tile([P, DOUT], F32, tag="bvrep")
    bv_bcast = bass.AP(
        tensor=bv_ap.tensor, offset=bv_ap.offset, ap=[[0, P]] + list(bv_ap.ap)
    )
    nc.gpsimd.dma_start(out=bv_rep, in_=bv_bcast)
    nc.gpsimd.dma_start(out=wk_sb[:, :, P:DOUT], in_=wkt_r[:, :, P:DOUT])
    nc.gpsimd.dma_start(out=wq_sb[:, :, P:DOUT], in_=wqt_r[:, :, P:DOUT])

    # ---- persistent activations (bf16) --------------------------------------
    # qT / kT: [dout, s] as 4 chunk-tiles of [128, 2048] (chunk = head pair)
    qT = [persist.tile([P, S], BF16, tag=f"qT{m}", name=f"qT{m}") for m in range(NM)]
    kT = [persist.tile([P, S], BF16, tag=f"kT{m}", name=f"kT{m}") for m in range(NM)]
    # v: [s, head, dk+1] tiles; col 64 of each head block holds ones so the
    # AV matmul's 65th output row accumulates the softmax denominator
    v_sb = [
        persist.tile([P, 8, 65], BF16, tag=f"v{i}", name=f"v{i}") for i in range(NK16)
    ]
    for i in range(NK16):
        nc.vector.memset(v_sb[i][:, :, 64:65], 1.0)
    # attn_outT: [dout, s] as 4 chunk-tiles (rows 0-63 even head, 64-127 odd)
    aoT = [persist.tile([P, S], BF16, tag=f"aoT{m}", name=f"aoT{m}") for m in range(NM)]

    # wo is only needed by the O-projection (pc=3) — DMA'd mid-attention
    wo_sb = persist.tile([P, NM, D], BF16, tag="wo")

    qin = {}

    def load_qin(si, eng=None):
        if si not in qin:
            qin[si] = qinp.tile([P, NKC, 512], BF16, tag="qin", name=f"qin{si}")
        (eng or nc.sync).dma_start(out=qin[si], in_=qt_r[:, :, ts(si, 512)])

    vin = {}

    def load_vin(g, eng=None):
        t = vinp.tile([P, NKC, 512], BF16, tag="vin", name=f"vin{g}")
        (eng or nc.sync).dma_start(out=t, in_=vt_r[:, :, ts(g, 512)])
        vin[g] = t

    # ---- projection chunk emitters ------------------------------------------
    def proj_chunk(w_sb, xin, m, dst, si, bias_base):
        """dst[m][:, si*512:(si+1)*512] = (W[m] @ x[si]) + b[m]  (8 MMs)."""
        ps = ps_pj.tile([P, 512], F32, tag="pj", name="pspj")
        for kc in range(NKC):
            nc.tensor.matmul(
                ps,
                lhsT=w_sb[:, kc, ts(m, P)],
                rhs=xin[:, kc, :],
                start=(kc == 0),
                stop=(kc == NKC - 1),
                skip_group_check=True,
            )
        c = bias_base + m
        nc.vector.tensor_add(
            out=dst[m][:, ts(si, 512)],
            in0=ps,
            in1=bqk_sb[:, c : c + 1].to_broadcast([P, 512]),
        )

    def vproj_block(g, j):
        """v_sb[4g+j][:, :, 0:64] = (v[si16] @ Wv^T) + bv  (8 MMs)."""
        si16 = g * 4 + j
        ps = ps_pj.tile([P, 512], F32, tag="pj", name="pspjv")
        for kc in range(NKC):
            nc.tensor.matmul(
                ps,
                lhsT=vin[g][:, kc, ts(j, P)],
                rhs=wv_sb[:, kc, :],
                start=(kc == 0),
                stop=(kc == NKC - 1),
                skip_group_check=True,
            )
        nc.vector.tensor_add(
            out=v_sb[si16][:, :, 0:64],
            in0=ps.rearrange("p (h d) -> p h d", h=8),
            in1=bv_rep.rearrange("p (h d) -> p h d", h=8),
        )

    # ---- prelude: unblock the attention pipeline ASAP -----------------------
    load_qin(0, eng=nc.scalar)
    load_vin(0, eng=nc.sync)
    # kT[0] si0 in two 256-key slabs so ST(ki=0,1) unblocks after slab 0
    for sl in range(2):
        ps = ps_pj.tile([P, 512], F32, tag="pj", name="pspj")
        for kc in range(NKC):
            nc.tensor.matmul(
                ps[:, 0:256],
                lhsT=wk_sb[:, kc, 0:P],
                rhs=kin0[:, kc, sl * 256 : sl * 256 + 256],
                start=(kc == 0),
                stop=(kc == NKC - 1),
                skip_group_check=True,
            )
        nc.vector.tensor_add(
            out=kT[0][:, sl * 256 : sl * 256 + 256],
            in0=ps[:, 0:256],
            in1=bqk_sb[:, NM : NM + 1].to_broadcast([P, 256]),
        )
    proj_chunk(wq_sb, qin[0], 0, qT, 0, 0)
    for j in range(4):
        vproj_block(0, j)

    # ---- spill queue: remaining projection work, deadline-paced -------------
    # Each entry is (deadline_iter, emit_fn); iter = pc*64 + qi*16 + ki of the
    # attention iteration that first consumes the chunk (with slack baked in).
    items = []

    def sp_k(m, si, dl, load=False):
        def emit(m=m, si=si, load=load):
            if load:
                load_kin(si)
            proj_chunk(wk_sb, kin[si], m, kT, si, NM)

        items.append((dl, emit))

    def sp_q(m, si, dl, load=False):
        def emit(m=m, si=si, load=load):
            if load:
                load_qin(si)
            proj_chunk(wq_sb, qin[si], m, qT, si, 0)

        items.append((dl, emit))

    def sp_v(g, j, dl):
        def emit(g=g, j=j):
            if j == 0:
                load_vin(g)
            vproj_block(g, j)

        items.append((dl, emit))

    for si in range(1, NQ):
        sp_k(0, si, 4 * si, load=True)
    for g in range(1, NQ):
        for j in range(4):
            sp_v(g, j, 4 * g + j + 4)
    sp_q(0, 1, 16, load=True)
    sp_q(0, 2, 32, load=True)
    # qT[1..3]@si0 must be emitted before the qin si3 load reuses its slot
    sp_q(1, 0, 40)
    sp_q(2, 0, 41)
    sp_q(3, 0, 42)
    sp_q(0, 3, 48, load=True)
    for m in range(1, NM):
        for si in range(NQ):
            sp_k(m, si, 64 * m + 4 * si)
    for m in range(1, NM):
        for si in range(1, NQ):
            sp_q(m, si, 64 * m + 16 * si)
    items.sort(key=lambda e: e[0])
    spill = deque(items)

    def drain(it):
        """Deadline-driven: emit chunks ~30 iters before first use (the spill
        matmuls only get TensorE's spare ~0.35us/iter, so a chunk needs many
        iterations of headroom); spread the surplus so filler work survives
        into the late ACT-bound phase."""
        hit = False
        while spill and spill[0][0] <= it + 30:
            spill.popleft()[1]()
            hit = True
        if (
            not hit
            and spill
            and it % 3 == 2
            and len(spill) > (4 * 64 - it) // 10
        ):
            spill.popleft()[1]()

    def drain_rest():
        while spill:
            spill.popleft()[1]()

    # ---- O-projection half-chunk (4 MMs; small enough not to displace exp) --
    def oproj_half(si16, n2):
        ps = ps_pj.tile([P, 512], F32, tag="pj", name="pso")
        for c in range(NM):
            nc.tensor.matmul(
                ps,
                lhsT=aoT[c][:, ts(si16, P)],
                rhs=wo_sb[:, c, ts(n2, 512)],
                start=(c == 0),
                stop=(c == NM - 1),
                skip_group_check=True,
            )
        osb = outp.tile([P, 512], F32, tag="osb")
        nc.vector.tensor_copy(out=osb, in_=ps)
        nc.gpsimd.dma_start(out=out_r[:, si16, ts(n2, 512)], in_=osb)

    def oproj_chunk(si16):
        oproj_half(si16, 0)
        oproj_half(si16, 1)

    # ---- attention (scores kept transposed: tiles are S^T[k, q]) ------------
    # head pair pc: even head = partitions/cols 0-63, odd head = 64-127
    for pc in range(NM):
        hh = 2 * pc
        if pc == 2:
            # load the O-projection weights well before pc=3 needs them
            # (sync queue: FIFO-staged behind the earlier input loads)
            nc.sync.dma_start(out=wo_sb, in_=wot_r)
        for qi in range(NQ):
            av_e = ps_av.tile([P, 512], F32, tag="av", name="av_e")  # rows 0-64
            av_o = ps_av.tile([P, 512], F32, tag="av", name="av_o")
            for ki in range(NK16):
                st = ps_st.tile([P, 1024], F32, tag="st", name="st")
                nc.tensor.matmul(
                    st[:, 0:512],
                    lhsT=kT[pc][0:64, ts(ki, P)],
                    rhs=qT[pc][0:64, ts(qi, 512)],
                    start=True,
                    stop=True,
                )
                nc.tensor.matmul(
                    st[:, 512:1024],
                    lhsT=kT[pc][64:128, ts(ki, P)],
                    rhs=qT[pc][64:128, ts(qi, 512)],
                    start=True,
                    stop=True,
                    skip_group_check=True,
                )
                et = etp.tile([P, 1024], BF16, tag="et", name="et")
                nc.scalar.activation(out=et, in_=st, func=FP.Exp, scale=0.125)
                first = ki == 0
                last = ki == NK16 - 1
                nc.tensor.matmul(
                    av_e[0:65],
                    lhsT=v_sb[ki][:, hh, :],
                    rhs=et[:, 0:512],
                    start=first,
                    stop=last,
                    skip_group_check=True,
                )
                nc.tensor.matmul(
                    av_o[0:65],
                    lhsT=v_sb[ki][:, hh + 1, :],
                    rhs=et[:, 512:1024],
                    start=first,
                    stop=last,
                    skip_group_check=True,
                )
                drain(64 * pc + 16 * qi + ki)
                # O-projection for the previous qi's s-range: six half-chunks
                # spread through the ki loop; the last chunk is reserved as a
                # filler bridging this qi's normalization latency
                if pc == NM - 1 and qi > 0 and ki % 2 == 1 and ki < 12:
                    oproj_half(4 * (qi - 1) + ki // 4, (ki // 2) % 2)
            # free the PSUM banks fast, then normalize from SBUF
            ae = avsb.tile([P, 512], F32, tag="ae", name="ae")
            ao = avsb.tile([P, 512], F32, tag="ae", name="ao")
            nc.vector.tensor_copy(out=ae[0:65], in_=av_e[0:65])
            nc.vector.tensor_copy(out=ao[0:65], in_=av_o[0:65])
            # denominators: bounce [2,512] to DRAM, reshape to [128,8] so the
            # iterative-divide reciprocal runs across partitions, not 1 lane
            scr = dramp.tile([2, 512], F32, tag="scr", name="scr")
            nc.sync.dma_start(out=scr[0:1, :], in_=ae[64:65, :])
            nc.sync.dma_start(out=scr[1:2, :], in_=ao[64:65, :])
            den = recipp.tile([P, 8], F32, tag="den", name="den")
            scr_flat = scr[:, :].rearrange("a b -> (a b)").rearrange(
                "(p f) -> p f", p=P
            )
            nc.sync.dma_start(out=den, in_=scr_flat)
            denr = recipp.tile([P, 8], F32, tag="denr", name="denr")
            nc.vector.reciprocal(out=denr, in_=den)
            scr2 = dramp.tile([P, 8], F32, tag="scr2", name="scr2")
            nc.sync.dma_start(out=scr2, in_=denr)
            rep = recipp.tile([P, 1024], F32, tag="rep", name="rep")
            s2f = scr2[:, :].rearrange("p f -> (p f)")
            nc.sync.dma_start(
                out=rep[0:64, :],
                in_=bass.AP(
                    tensor=s2f.tensor, offset=s2f.offset, ap=[[0, 64]] + list(s2f.ap)
                ),
            )
            last_block = pc == NM - 1 and qi == NQ - 1
            if not last_block:
                nc.vector.tensor_mul(
                    out=aoT[pc][0:64, ts(qi, 512)], in0=ae[0:64], in1=rep[0:64, 0:512]
                )
                stag = stagp.tile([P, 512], BF16, tag="stag", name="stag")
                nc.vector.tensor_mul(
                    out=stag[0:64, :], in0=ao[0:64], in1=rep[0:64, 512:1024]
                )
                nc.sync.dma_start(
                    out=aoT[pc][64:128, ts(qi, 512)], in_=stag[0:64, :]
                )
                if pc == NM - 1 and qi > 0:
                    # reserved filler: keep TensorE warm behind the norm chain
                    oproj_chunk(4 * (qi - 1) + 3)
            else:
                # final block: normalize in 128-col pieces, each immediately
                # feeding its O-projection chunk so the tail pipelines;
                # qi=2's last chunk bridges the reciprocal chain latency
                oproj_chunk(4 * (qi - 1) + 3)
                for piece in range(4):
                    cs = qi * 512 + piece * 128
                    ps_lo = slice(piece * 128, piece * 128 + 128)
                    ps_hi = slice(512 + piece * 128, 512 + piece * 128 + 128)
                    nc.vector.tensor_mul(
                        out=aoT[pc][0:64, cs : cs + 128],
                        in0=ae[0:64, piece * 128 : piece * 128 + 128],
                        in1=rep[0:64, ps_lo],
                    )
                    stag = stagp.tile([P, 128], BF16, tag="stagf", name="stagf")
                    nc.vector.tensor_mul(
                        out=stag[0:64, :],
                        in0=ao[0:64, piece * 128 : piece * 128 + 128],
                        in1=rep[0:64, ps_hi],
                    )
                    nc.sync.dma_start(
                        out=aoT[pc][64:128, cs : cs + 128], in_=stag[0:64, :]
                    )
                    oproj_chunk(4 * qi + piece)

    drain_rest()


def _build():
    global _cached_nc
    if _cached_nc is not None:
        return _cached_nc
    nc = bacc.Bacc("TRN2", target_bir_lowering=False, debug=False)
    io = {
        "qt": nc.dram_tensor("qt", [D, S], BF16, kind="ExternalInput"),
        "kt": nc.dram_tensor("kt", [D, S], BF16, kind="ExternalInput"),
        "vt": nc.dram_tensor("vt", [D, S], BF16, kind="ExternalInput"),
        "wqt": nc.dram_tensor("wqt", [D, DOUT], BF16, kind="ExternalInput"),
        "wkt": nc.dram_tensor("wkt", [D, DOUT], BF16, kind="ExternalInput"),
        "wvt": nc.dram_tensor("wvt", [D, DOUT], BF16, kind="ExternalInput"),
        "wot": nc.dram_tensor("wot", [DOUT, D], BF16, kind="ExternalInput"),
        "bqk": nc.dram_tensor("bqk", [2 * DOUT], F32, kind="ExternalInput"),
        "bv": nc.dram_tensor("bv", [DOUT], F32, kind="ExternalInput"),
        "out": nc.dram_tensor("out", [S, D], F32, kind="ExternalOutput"),
    }
    with tile.TileContext(nc) as tc:
        with ExitStack() as ctx:
            _emit(ctx, tc, io)
    nc.compile()
    _cached_nc = nc
    return nc


def make_in_maps(Q, K, V, Wq, bq, Wk, bk, Wv, bv, Wo):
    bf = lambda a: np.ascontiguousarray(np.asarray(a, np.float32)).astype(
        ml_dtypes.bfloat16
    )
    f = lambda a: np.ascontiguousarray(a, dtype=np.float32)
    in_maps = []
    for c in range(N_CORES):
        b = c // 2
        lo = (c % 2) * DOUT
        sl = slice(lo, lo + DOUT)
        in_maps.append(
            {
                "qt": bf(np.asarray(Q, np.float32)[b].T),
                "kt": bf(np.asarray(K, np.float32)[b].T),
                "vt": bf(np.asarray(V, np.float32)[b].T),
                "wqt": bf(np.asarray(Wq, np.float32)[sl, :].T),
                "wkt": bf(np.asarray(Wk, np.float32)[sl, :].T),
                "wvt": bf(np.asarray(Wv, np.float32)[sl, :].T),
                "wot": bf(np.asarray(Wo, np.float32)[:, sl].T),
                # partition-major shuffle: bqk[p, c] = bias[c*128 + p]
                "bqk": f(
                    np.concatenate(
                        [
                            np.asarray(bq[sl]).reshape(NM, P).T,
                            np.asarray(bk[sl]).reshape(NM, P).T,
                        ],
                        axis=1,
                    ).reshape(-1)
                ),
                "bv": f(bv[sl]),
            }
        )
    return in_maps


def gather_output(results, bo):
    out = np.empty((B, S, D), dtype=np.float32)
    bo = np.asarray(bo, dtype=np.float32)
    for b in range(B):
        out[b] = results[2 * b]["out"] + results[2 * b + 1]["out"] + bo
    return out


def _numpy_fallback(Q, K, V, mask, Wq, bq, Wk, bk, Wv, bv, Wo, bo):
    """Exact reference math in numpy (only used if mask isn't all-ones)."""
    H, dk = 16, 64
    out = np.empty((B, S, D), dtype=np.float32)
    for b in range(B):
        q = (Q[b] @ Wq.T + bq).reshape(S, H, dk).transpose(1, 0, 2)
        k = (K[b] @ Wk.T + bk).reshape(S, H, dk).transpose(1, 0, 2)
        v = (V[b] @ Wv.T + bv).reshape(S, H, dk).transpose(1, 0, 2)
        o = np.empty((H, S, dk), dtype=np.float32)
        for h in range(H):
            s = (q[h] @ k[h].T) / np.sqrt(np.float32(dk))
            s = np.where(mask[b] == 0, np.float32(-1.0e9), s)
            s = s - s.max(axis=-1, keepdims=True)
            e = np.exp(s)
            a = e / e.sum(axis=-1, keepdims=True)
            o[h] = a @ v[h]
        out[b] = o.transpose(1, 0, 2).reshape(S, H * dk) @ Wo.T + bo
    return out


def kernel(Q, K, V, mask, Wq, bq, Wk, bk, Wv, bv, Wo, bo):
    Q = np.asarray(Q, dtype=np.float32)
    K = np.asarray(K, dtype=np.float32)
    V = np.asarray(V, dtype=np.float32)
    Wq = np.asarray(Wq, dtype=np.float32)
    Wk = np.asarray(Wk, dtype=np.float32)
    Wv = np.asarray(Wv, dtype=np.float32)
    Wo = np.asarray(Wo, dtype=np.float32)
    bq = np.asarray(bq, dtype=np.float32)
    bk = np.asarray(bk, dtype=np.float32)
    bv = np.asarray(bv, dtype=np.float32)
    bo = np.asarray(bo, dtype=np.float32)
    mask_np = np.asarray(mask)

    if not np.all(mask_np != 0):
        return _numpy_fallback(Q, K, V, mask_np, Wq, bq, Wk, bk, Wv, bv, Wo, bo)

    nc = _build()
    in_maps = make_in_maps(Q, K, V, Wq, bq, Wk, bk, Wv, bv, Wo)
    res = run_bass_kernel_spmd(nc, in_maps, list(range(N_CORES))).results
    return gather_output(res, bo)


# revision 56
# speedup vs baseline: 1.0206x; 1.0087x over previous
"""Multi-head attention (B=4, S=2048, D=1024, H=16) on 8 Trainium2 cores.

Sharding: data-parallel over batch (4) x tensor-parallel over heads (2).
Core c handles batch c//2 and heads (c%2)*8 .. +8.  Each core computes a
partial output (its heads' contribution through the O-projection); the host
sums the two partials per batch and adds the output bias.

Single fused pipeline: the softmax exp stream on ScalarE (33.5M elem/core,
~1 elem/cycle/lane) is the hard floor, so the attention loop starts after a
minimal prelude (kT[0]/qT[0] first chunks + v head chunk 0) and the rest of
the q/k/v projections plus the O-projection are drained into TensorE's spare
capacity through a spill queue interleaved with the attention iterations.
Scores stay transposed (S^T[k,q]); the softmax denominator rides the AV
matmul as a 65th V column of ones; the denominator reciprocal runs on a
[128,8]-reshaped view (cheap) instead of a single-partition [1,512] vector.
"""

import numpy as np
from collections import deque
from contextlib import ExitStack

import ml_dtypes
import concourse.bass as bass
import concourse.tile as tile
from concourse import bacc, mybir
from concourse.bass import ts
from concourse.bass_utils import run_bass_kernel_spmd

P = 128
S = 2048          # sequence length
D = 1024          # model dim
DOUT = 512        # per-core projection width (8 heads x 64)
DK = 64           # head dim
B = 4
N_CORES = 8
F32 = mybir.dt.float32
BF16 = mybir.dt.bfloat16
FP = mybir.ActivationFunctionType

NKC = D // P      # 8 contraction chunks over model dim
NM = DOUT // P    # 4 dout chunks (also head pairs)
NQ = S // 512     # 4 query chunks of 512
NK16 = S // P     # 16 key chunks of 128

_cached_nc = None


def _emit(ctx: ExitStack, tc: "tile.TileContext", io: dict):
    nc = tc.nc

    qt_r = io["qt"].ap().rearrange("(c p) s -> p c s", p=P)      # [128, 8, 2048]
    kt_r = io["kt"].ap().rearrange("(c p) s -> p c s", p=P)
    vt_r = io["vt"].ap().rearrange("(c p) s -> p c s", p=P)
    wqt_r = io["wqt"].ap().rearrange("(c p) m -> p c m", p=P)    # [128, 8, 512]
    wkt_r = io["wkt"].ap().rearrange("(c p) m -> p c m", p=P)
    wvt_r = io["wvt"].ap().rearrange("(c p) m -> p c m", p=P)
    wot_r = io["wot"].ap().rearrange("(c p) n -> p c n", p=P)    # [128, 4, 1024]
    # host supplies bq/bk pre-shuffled to partition-major [128, 4] each so the
    # DMA is one clean burst (the naive [512]->(c p) pattern is 4-byte strided)
    bqk_r = io["bqk"].ap().rearrange("(p c) -> p c", p=P)        # [128, 8]
    bv_ap = io["bv"].ap()                                        # [512]
    out_r = io["out"].ap().rearrange("(sc p) n -> p sc n", p=P)  # [128, 16, 1024]

    persist = ctx.enter_context(tc.tile_pool(name="persist", bufs=1))
    kinp = ctx.enter_context(tc.tile_pool(name="kinp", bufs=4))
    qinp = ctx.enter_context(tc.tile_pool(name="qinp", bufs=3))
    vinp = ctx.enter_context(tc.tile_pool(name="vinp", bufs=2))
    etp = ctx.enter_context(tc.tile_pool(name="etp", bufs=8))
    avsb = ctx.enter_context(tc.tile_pool(name="avsb", bufs=3))
    recipp = ctx.enter_context(tc.tile_pool(name="recipp", bufs=2))
    stagp = ctx.enter_context(tc.tile_pool(name="stagp", bufs=1))
    outp = ctx.enter_context(tc.tile_pool(name="outp", bufs=2))
    dramp = ctx.enter_context(tc.tile_pool(name="dramp", bufs=3, space="DRAM"))

    ps_st = ctx.enter_context(tc.tile_pool(name="ps_st", bufs=2, space="PSUM"))
    ps_av = ctx.enter_context(tc.tile_pool(name="ps_av", bufs=2, space="PSUM"))
    ps_pj = ctx.enter_context(tc.tile_pool(name="ps_pj", bufs=2, space="PSUM"))

    # ---- prelude-critical DMAs, bandwidth-staged ----------------------------
    # Wave 1 (parallel queues): kin si0 (sync), wk (gpsimd), qin si0 (scalar).
    # Wave 2: vin g0 (sync), wv (gpsimd), wq (scalar).  Everything else (kin
    # si1-3, wo, qin si1-3, vin g1-3) is deferred so it doesn't steal HBM
    # bandwidth from the pipeline-critical prelude bytes.
    kin = {}

    def load_kin(si, eng=None):
        t = kinp.tile([P, NKC, 512], BF16, tag="kin", name=f"kin{si}")
        (eng or nc.sync).dma_start(out=t, in_=kt_r[:, :, ts(si, 512)])
        kin[si] = t

    # kin si0 in two key-slabs (256 keys each) so the first kT matmuls start
    # after only 512KB of input bytes
    kin0 = kinp.tile([P, NKC, 512], BF16, tag="kin", name="kin0")
    nc.sync.dma_start(out=kin0[:, :, 0:256], in_=kt_r[:, :, 0:256])
    nc.sync.dma_start(out=kin0[:, :, 256:512], in_=kt_r[:, :, 256:512])
    kin[0] = kin0
    # weight loads split by m-columns: the whole pc=0 pipeline (kT[0]/qT[0])
    # only touches the m=0 column slice (256KB); the rest streams later
    wk_sb = persist.tile([P, NKC, DOUT], BF16, tag="wk")
    nc.gpsimd.dma_start(out=wk_sb[:, :, 0:P], in_=wkt_r[:, :, 0:P])
    bqk_sb = persist.tile([P, 2 * NM], F32, tag="bqk")
    nc.gpsimd.dma_start(out=bqk_sb, in_=bqk_r)
    wq_sb = persist.tile([P, NKC, DOUT], BF16, tag="wq")
    nc.gpsimd.dma_start(out=wq_sb[:, :, 0:P], in_=wqt_r[:, :, 0:P])
    wv_sb = persist.tile([P, NKC, DOUT], BF16, tag="wv")
    nc.gpsimd.dma_start(out=wv_sb, in_=wvt_r)
    bv_rep = persist.tile([P, DOUT], F32, tag="bvrep")
    bv_bcast = bass.AP(
        tensor=bv_ap.tensor, offset=bv_ap.offset, ap=[[0, P]] + list(bv_ap.ap)
    )
    nc.gpsimd.dma_start(out=bv_rep, in_=bv_bcast)
    nc.gpsimd.dma_start(out=wk_sb[:, :, P:DOUT], in_=wkt_r[:, :, P:DOUT])
    nc.gpsimd.dma_start(out=wq_sb[:, :, P:DOUT], in_=wqt_r[:, :, P:DOUT])

    # ---- persistent activations (bf16) --------------------------------------
    # qT / kT: [dout, s] as 4 chunk-tiles of [128, 2048] (chunk = head pair)
    qT = [persist.tile([P, S], BF16, tag=f"qT{m}", name=f"qT{m}") for m in range(NM)]
    kT = [persist.tile([P, S], BF16, tag=f"kT{m}", name=f"kT{m}") for m in range(NM)]
    # v: [s, head, dk+1] tiles; col 64 of each head block holds ones so the
    # AV matmul's 65th output row accumulates the softmax denominator
    v_sb = [
        persist.tile([P, 8, 65], BF16, tag=f"v{i}", name=f"v{i}") for i in range(NK16)
    ]
    for i in range(NK16):
        nc.vector.memset(v_sb[i][:, :, 64:65], 1.0)
    # attn_outT: [dout, s] as 4 chunk-tiles (rows 0-63 even head, 64-127 odd)
    aoT = [persist.tile([P, S], BF16, tag=f"aoT{m}", name=f"aoT{m}") for m in range(NM)]

    # wo is only needed by the O-projection (pc=3) — DMA'd mid-attention
    wo_sb = persist.tile([P, NM, D], BF16, tag="wo")

    qin = {}

    def load_qin(si, eng=None):
        if si not in qin:
            qin[si] = qinp.tile([P, NKC, 512], BF16, tag="qin", name=f"qin{si}")
        (eng or nc.sync).dma_start(out=qin[si], in_=qt_r[:, :, ts(si, 512)])

    vin = {}

    def load_vin(g, eng=None):
        t = vinp.tile([P, NKC, 512], BF16, tag="vin", name=f"vin{g}")
        (eng or nc.sync).dma_start(out=t, in_=vt_r[:, :, ts(g, 512)])
        vin[g] = t

    # ---- projection chunk emitters ------------------------------------------
    def proj_chunk(w_sb, xin, m, dst, si, bias_base):
        """dst[m][:, si*512:(si+1)*512] = (W[m] @ x[si]) + b[m]  (8 MMs)."""
        ps = ps_pj.tile([P, 512], F32, tag="pj", name="pspj")
        for kc in range(NKC):
            nc.tensor.matmul(
                ps,
                lhsT=w_sb[:, kc, ts(m, P)],
                rhs=xin[:, kc, :],
                start=(kc == 0),
                stop=(kc == NKC - 1),
                skip_group_check=True,
            )
        c = bias_base + m
        nc.vector.tensor_add(
            out=dst[m][:, ts(si, 512)],
            in0=ps,
            in1=bqk_sb[:, c : c + 1].to_broadcast([P, 512]),
        )

    def vproj_block(g, j):
        """v_sb[4g+j][:, :, 0:64] = (v[si16] @ Wv^T) + bv  (8 MMs)."""
        si16 = g * 4 + j
        ps = ps_pj.tile([P, 512], F32, tag="pj", name="pspjv")
        for kc in range(NKC):
            nc.tensor.matmul(
                ps,
                lhsT=vin[g][:, kc, ts(j, P)],
                rhs=wv_sb[:, kc, :],
                start=(kc == 0),
                stop=(kc == NKC - 1),
                skip_group_check=True,
            )
        nc.vector.tensor_add(
            out=v_sb[si16][:, :, 0:64],
            in0=ps.rearrange("p (h d) -> p h d", h=8),
            in1=bv_rep.rearrange("p (h d) -> p h d", h=8),
        )

    # ---- prelude: unblock the attention pipeline ASAP -----------------------
    load_qin(0, eng=nc.scalar)
    load_vin(0, eng=nc.sync)
    # kT[0] si0 in two 256-key slabs so ST(ki=0,1) unblocks after slab 0
    for sl in range(2):
        ps = ps_pj.tile([P, 512], F32, tag="pj", name="pspj")
        for kc in range(NKC):
            nc.tensor.matmul(
                ps[:, 0:256],
                lhsT=wk_sb[:, kc, 0:P],
                rhs=kin0[:, kc, sl * 256 : sl * 256 + 256],
                start=(kc == 0),
                stop=(kc == NKC - 1),
                skip_group_check=True,
            )
        nc.vector.tensor_add(
            out=kT[0][:, sl * 256 : sl * 256 + 256],
            in0=ps[:, 0:256],
            in1=bqk_sb[:, NM : NM + 1].to_broadcast([P, 256]),
        )
    proj_chunk(wq_sb, qin[0], 0, qT, 0, 0)
    for j in range(4):
        vproj_block(0, j)

    # ---- spill queue: remaining projection work, deadline-paced -------------
    # Each entry is (deadline_iter, emit_fn); iter = pc*64 + qi*16 + ki of the
    # attention iteration that first consumes the chunk (with slack baked in).
    items = []

    def sp_k(m, si, dl, load=False):
        def emit(m=m, si=si, load=load):
            if load:
                load_kin(si)
            proj_chunk(wk_sb, kin[si], m, kT, si, NM)

        items.append((dl, emit))

    def sp_q(m, si, dl, load=False):
        def emit(m=m, si=si, load=load):
            if load:
                load_qin(si)
            proj_chunk(wq_sb, qin[si], m, qT, si, 0)

        items.append((dl, emit))

    def sp_v(g, j, dl):
        def emit(g=g, j=j):
            if j == 0:
                load_vin(g)
            vproj_block(g, j)

        items.append((dl, emit))

    for si in range(1, NQ):
        sp_k(0, si, 4 * si, load=True)
    for g in range(1, NQ):
        for j in range(4):
            sp_v(g, j, 4 * g + j + 4)
    sp_q(0, 1, 16, load=True)
    sp_q(0, 2, 32, load=True)
    # qT[1..3]@si0 must be emitted before the qin si3 load reuses its slot
    sp_q(1, 0, 40)
    sp_q(2, 0, 41)
    sp_q(3, 0, 42)
    sp_q(0, 3, 48, load=True)
    for m in range(1, NM):
        for si in range(NQ):
            sp_k(m, si, 64 * m + 4 * si)
    for m in range(1, NM):
        for si in range(1, NQ):
            sp_q(m, si, 64 * m + 16 * si)
    items.sort(key=lambda e: e[0])
    spill = deque(items)

    def drain(it):
        """Deadline-driven: emit chunks ~30 iters before first use (the spill
        matmuls only get TensorE's spare ~0.35us/iter, so a chunk needs many
        iterations of headroom); spread the surplus so filler work survives
        into the late ACT-bound phase."""
        hit = False
        while spill and spill[0][0] <= it + 30:
            spill.popleft()[1]()
            hit = True
        if (
            not hit
            and spill
            and it % 3 == 2
            and len(spill) > (4 * 64 - it) // 10
        ):
            spill.popleft()[1]()

    def drain_rest():
        while spill:
            spill.popleft()[1]()

    # ---- O-projection half-chunk (4 MMs; small enough not to displace exp) --
    def oproj_half(si16, n2):
        ps = ps_pj.tile([P, 512], F32, tag="pj", name="pso")
        for c in range(NM):
            nc.tensor.matmul(
                ps,
                lhsT=aoT[c][:, ts(si16, P)],
                rhs=wo_sb[:, c, ts(n2, 512)],
                start=(c == 0),
                stop=(c == NM - 1),
                skip_group_check=True,
            )
        osb = outp.tile([P, 512], F32, tag="osb")
        nc.vector.tensor_copy(out=osb, in_=ps)
        nc.gpsimd.dma_start(out=out_r[:, si16, ts(n2, 512)], in_=osb)

    def oproj_chunk(si16):
        oproj_half(si16, 0)
        oproj_half(si16, 1)

    # ---- attention (scores kept transposed: tiles are S^T[k, q]) ------------
    # head pair pc: even head = partitions/cols 0-63, odd head = 64-127
    for pc in range(NM):
        hh = 2 * pc
        if pc == 2:
            # load the O-projection weights well before pc=3 needs them
            # (sync queue: FIFO-staged behind the earlier input loads)
            nc.sync.dma_start(out=wo_sb, in_=wot_r)
        for qi in range(NQ):
            av_e = ps_av.tile([P, 512], F32, tag="av", name="av_e")  # rows 0-64
            av_o = ps_av.tile([P, 512], F32, tag="av", name="av_o")
            for ki in range(NK16):
                st = ps_st.tile([P, 1024], F32, tag="st", name="st")
                nc.tensor.matmul(
                    st[:, 0:512],
                    lhsT=kT[pc][0:64, ts(ki, P)],
                    rhs=qT[pc][0:64, ts(qi, 512)],
                    start=True,
                    stop=True,
                )
                nc.tensor.matmul(
                    st[:, 512:1024],
                    lhsT=kT[pc][64:128, ts(ki, P)],
                    rhs=qT[pc][64:128, ts(qi, 512)],
                    start=True,
                    stop=True,
                    skip_group_check=True,
                )
                et = etp.tile([P, 1024], BF16, tag="et", name="et")
                nc.scalar.activation(out=et, in_=st, func=FP.Exp, scale=0.125)
                first = ki == 0
                last = ki == NK16 - 1
                nc.tensor.matmul(
                    av_e[0:65],
                    lhsT=v_sb[ki][:, hh, :],
                    rhs=et[:, 0:512],
                    start=first,
                    stop=last,
                    skip_group_check=True,
                )
                nc.tensor.matmul(
                    av_o[0:65],
                    lhsT=v_sb[ki][:, hh + 1, :],
                    rhs=et[:, 512:1024],
                    start=first,
                    stop=last,
                    skip_group_check=True,
                )
                drain(64 * pc + 16 * qi + ki)
                # O-projection for the previous qi's s-range: six half-chunks
                # spread through the ki loop; the last chunk is reserved as a
                # filler bridging this qi's normalization latency
                if pc == NM - 1 and qi > 0 and ki % 2 == 1 and ki < 12:
                    oproj_half(4 * (qi - 1) + ki // 4, (ki // 2) % 2)
            # free the PSUM banks fast, then normalize from SBUF
            ae = avsb.tile([P, 512], F32, tag="ae", name="ae")
            ao = avsb.tile([P, 512], F32, tag="ae", name="ao")
            nc.vector.tensor_copy(out=ae[0:65], in_=av_e[0:65])
            nc.vector.tensor_copy(out=ao[0:65], in_=av_o[0:65])
            # denominators: bounce [2,512] to DRAM, reshape to [128,8] so the
            # iterative-divide reciprocal runs across partitions, not 1 lane
            scr = dramp.tile([2, 512], F32, tag="scr", name="scr")
            nc.sync.dma_start(out=scr[0:1, :], in_=ae[64:65, :])
            nc.sync.dma_start(out=scr[1:2, :], in_=ao[64:65, :])
            den = recipp.tile([P, 8], F32, tag="den", name="den")
            scr_flat = scr[:, :].rearrange("a b -> (a b)").rearrange(
                "(p f) -> p f", p=P
            )
            nc.sync.dma_start(out=den, in_=scr_flat)
            denr = recipp.tile([P, 8], F32, tag="denr", name="denr")
            nc.vector.reciprocal(out=denr, in_=den)
            scr2 = dramp.tile([P, 8], F32, tag="scr2", name="scr2")
            nc.sync.dma_start(out=scr2, in_=denr)
            rep = recipp.tile([P, 1024], F32, tag="rep", name="rep")
            s2f = scr2[:, :].rearrange("p f -> (p f)")
            nc.sync.dma_start(
                out=rep[0:64, :],
                in_=bass.AP(
                    tensor=s2f.tensor, offset=s2f.offset, ap=[[0, 64]] + list(s2f.ap)
                ),
            )
            last_block = pc == NM - 1 and qi == NQ - 1
            if not last_block:
                nc.vector.tensor_mul(
                    out=aoT[pc][0:64, ts(qi, 512)], in0=ae[0:64], in1=rep[0:64, 0:512]
                )
                stag = stagp.tile([P, 512], BF16, tag="stag", name="stag")
                nc.vector.tensor_mul(
                    out=stag[0:64, :], in0=ao[0:64], in1=rep[0:64, 512:1024]
                )
                nc.sync.dma_start(
                    out=aoT[pc][64:128, ts(qi, 512)], in_=stag[0:64, :]
                )
                if pc == NM - 1 and qi > 0:
                    # reserved filler: keep TensorE warm behind the norm chain
                    oproj_chunk(4 * (qi - 1) + 3)
            else:
                # final block: normalize in 128-col pieces, each immediately
                # feeding its O-projection chunk so the tail pipelines;
                # qi=2's last chunk bridges the reciprocal chain latency
                oproj_chunk(4 * (qi - 1) + 3)
                for piece in range(4):
                    cs = qi * 512 + piece * 128
                    ps_lo = slice(piece * 128, piece * 128 + 128)
                    ps_hi = slice(512 + piece * 128, 512 + piece * 128 + 128)
                    nc.vector.tensor_mul(
                        out=aoT[pc][0:64, cs : cs + 128],
                        in0=ae[0:64, piece * 128 : piece * 128 + 128],
                        in1=rep[0:64, ps_lo],
                    )
                    stag = stagp.tile([P, 128], BF16, tag="stagf", name="stagf")
                    nc.vector.tensor_mul(
                        out=stag[0:64, :],
                        in0=ao[0:64, piece * 128 : piece * 128 + 128],
                        in1=rep[0:64, ps_hi],
                    )
                    nc.sync.dma_start(
                        out=aoT[pc][64:128, cs : cs + 128], in_=stag[0:64, :]
                    )
                    oproj_chunk(4 * qi + piece)

    drain_rest()


def _build():
    global _cached_nc
    if _cached_nc is not None:
        return _cached_nc
    nc = bacc.Bacc("TRN2", target_bir_lowering=False, debug=False)
    io = {
        "qt": nc.dram_tensor("qt", [D, S], BF16, kind="ExternalInput"),
        "kt": nc.dram_tensor("kt", [D, S], BF16, kind="ExternalInput"),
        "vt": nc.dram_tensor("vt", [D, S], BF16, kind="ExternalInput"),
        "wqt": nc.dram_tensor("wqt", [D, DOUT], BF16, kind="ExternalInput"),
        "wkt": nc.dram_tensor("wkt", [D, DOUT], BF16, kind="ExternalInput"),
        "wvt": nc.dram_tensor("wvt", [D, DOUT], BF16, kind="ExternalInput"),
        "wot": nc.dram_tensor("wot", [DOUT, D], BF16, kind="ExternalInput"),
        "bqk": nc.dram_tensor("bqk", [2 * DOUT], F32, kind="ExternalInput"),
        "bv": nc.dram_tensor("bv", [DOUT], F32, kind="ExternalInput"),
        "out": nc.dram_tensor("out", [S, D], F32, kind="ExternalOutput"),
    }
    with tile.TileContext(nc) as tc:
        with ExitStack() as ctx:
            _emit(ctx, tc, io)
    nc.compile()
    _cached_nc = nc
    return nc


def make_in_maps(Q, K, V, Wq, bq, Wk, bk, Wv, bv, Wo):
    bf = lambda a: np.ascontiguousarray(np.asarray(a, np.float32)).astype(
        ml_dtypes.bfloat16
    )
    f = lambda a: np.ascontiguousarray(a, dtype=np.float32)
    in_maps = []
    for c in range(N_CORES):
        b = c // 2
        lo = (c % 2) * DOUT
        sl = slice(lo, lo + DOUT)
        in_maps.append(
            {
                "qt": bf(np.asarray(Q, np.float32)[b].T),
                "kt": bf(np.asarray(K, np.float32)[b].T),
                "vt": bf(np.asarray(V, np.float32)[b].T),
                "wqt": bf(np.asarray(Wq, np.float32)[sl, :].T),
                "wkt": bf(np.asarray(Wk, np.float32)[sl, :].T),
                "wvt": bf(np.asarray(Wv, np.float32)[sl, :].T),
                "wot": bf(np.asarray(Wo, np.float32)[:, sl].T),
                # partition-major shuffle: bqk[p, c] = bias[c*128 + p]
                "bqk": f(
                    np.concatenate(
                        [
                            np.asarray(bq[sl]).reshape(NM, P).T,
                            np.asarray(bk[sl]).reshape(NM, P).T,
                        ],
                        axis=1,
                    ).reshape(-1)
                ),
                "bv": f(bv[sl]),
            }
        )
    return in_maps


def gather_output(results, bo):
    out = np.empty((B, S, D), dtype=np.float32)
    bo = np.asarray(bo, dtype=np.float32)
    for b in range(B):
        out[b] = results[2 * b]["out"] + results[2 * b + 1]["out"] + bo
    return out


def _numpy_fallback(Q, K, V, mask, Wq, bq, Wk, bk, Wv, bv, Wo, bo):
    """Exact reference math in numpy (only used if mask isn't all-ones)."""
    H, dk = 16, 64
    out = np.empty((B, S, D), dtype=np.float32)
    for b in range(B):
        q = (Q[b] @ Wq.T + bq).reshape(S, H, dk).transpose(1, 0, 2)
        k = (K[b] @ Wk.T + bk).reshape(S, H, dk).transpose(1, 0, 2)
        v = (V[b] @ Wv.T + bv).reshape(S, H, dk).transpose(1, 0, 2)
        o = np.empty((H, S, dk), dtype=np.float32)
        for h in range(H):
            s = (q[h] @ k[h].T) / np.sqrt(np.float32(dk))
            s = np.where(mask[b] == 0, np.float32(-1.0e9), s)
            s = s - s.max(axis=-1, keepdims=True)
            e = np.exp(s)
            a = e / e.sum(axis=-1, keepdims=True)
            o[h] = a @ v[h]
        out[b] = o.transpose(1, 0, 2).reshape(S, H * dk) @ Wo.T + bo
    return out


def kernel(Q, K, V, mask, Wq, bq, Wk, bk, Wv, bv, Wo, bo):
    Q = np.asarray(Q, dtype=np.float32)
    K = np.asarray(K, dtype=np.float32)
    V = np.asarray(V, dtype=np.float32)
    Wq = np.asarray(Wq, dtype=np.float32)
    Wk = np.asarray(Wk, dtype=np.float32)
    Wv = np.asarray(Wv, dtype=np.float32)
    Wo = np.asarray(Wo, dtype=np.float32)
    bq = np.asarray(bq, dtype=np.float32)
    bk = np.asarray(bk, dtype=np.float32)
    bv = np.asarray(bv, dtype=np.float32)
    bo = np.asarray(bo, dtype=np.float32)
    mask_np = np.asarray(mask)

    if not np.all(mask_np != 0):
        return _numpy_fallback(Q, K, V, mask_np, Wq, bq, Wk, bk, Wv, bv, Wo, bo)

    nc = _build()
    in_maps = make_in_maps(Q, K, V, Wq, bq, Wk, bk, Wv, bv, Wo)
    res = run_bass_kernel_spmd(nc, in_maps, list(range(N_CORES))).results
    return gather_output(res, bo)


# revision 59
# speedup vs baseline: 1.0470x; 1.0259x over previous
"""Multi-head attention (B=4, S=2048, D=1024, H=16) on 8 Trainium2 cores.

Sharding: data-parallel over batch (4) x tensor-parallel over heads (2).
Core c handles batch c//2 and heads (c%2)*8 .. +8.  Each core computes a
partial output (its heads' contribution through the O-projection); the host
sums the two partials per batch and adds the output bias.

Single fused pipeline: the softmax exp stream on ScalarE (33.5M elem/core,
~1 elem/cycle/lane) is the hard floor, so the attention loop starts after a
minimal prelude (kT[0]/qT[0] first chunks + v head chunk 0) and the rest of
the q/k/v projections plus the O-projection are drained into TensorE's spare
capacity through a spill queue interleaved with the attention iterations.
Scores stay transposed (S^T[k,q]); the softmax denominator rides the AV
matmul as a 65th V column of ones; the denominator reciprocal runs on a
[128,8]-reshaped view (cheap) instead of a single-partition [1,512] vector.
"""

import numpy as np
from collections import deque
from contextlib import ExitStack

import ml_dtypes
import concourse.bass as bass
import concourse.tile as tile
from concourse import bacc, mybir
from concourse.bass import ts
from concourse.bass_utils import run_bass_kernel_spmd

P = 128
S = 2048          # sequence length
D = 1024          # model dim
DOUT = 512        # per-core projection width (8 heads x 64)
DK = 64           # head dim
B = 4
N_CORES = 8
F32 = mybir.dt.float32
BF16 = mybir.dt.bfloat16
FP = mybir.ActivationFunctionType

NKC = D // P      # 8 contraction chunks over model dim
NM = DOUT // P    # 4 dout chunks (also head pairs)
NQ = S // 512     # 4 query chunks of 512
NK16 = S // P     # 16 key chunks of 128

_cached_nc = None


def _emit(ctx: ExitStack, tc: "tile.TileContext", io: dict):
    nc = tc.nc

    qt_r = io["qt"].ap().rearrange("(c p) s -> p c s", p=P)      # [128, 8, 2048]
    kt_r = io["kt"].ap().rearrange("(c p) s -> p c s", p=P)
    vt_r = io["vt"].ap().rearrange("(c p) s -> p c s", p=P)
    wqt_r = io["wqt"].ap().rearrange("(c p) m -> p c m", p=P)    # [128, 8, 512]
    wkt_r = io["wkt"].ap().rearrange("(c p) m -> p c m", p=P)
    wvt_r = io["wvt"].ap().rearrange("(c p) m -> p c m", p=P)
    wot_r = io["wot"].ap().rearrange("(c p) n -> p c n", p=P)    # [128, 4, 1024]
    # host supplies bq/bk pre-shuffled to partition-major [128, 4] each so the
    # DMA is one clean burst (the naive [512]->(c p) pattern is 4-byte strided)
    bqk_r = io["bqk"].ap().rearrange("(p c) -> p c", p=P)        # [128, 8]
    bv_ap = io["bv"].ap()                                        # [512]
    out_r = io["out"].ap().rearrange("(sc p) n -> p sc n", p=P)  # [128, 16, 1024]

    persist = ctx.enter_context(tc.tile_pool(name="persist", bufs=1))
    kinp = ctx.enter_context(tc.tile_pool(name="kinp", bufs=4))
    qinp = ctx.enter_context(tc.tile_pool(name="qinp", bufs=3))
    vinp = ctx.enter_context(tc.tile_pool(name="vinp", bufs=2))
    etp = ctx.enter_context(tc.tile_pool(name="etp", bufs=7))
    avsb = ctx.enter_context(tc.tile_pool(name="avsb", bufs=3))
    recipp = ctx.enter_context(tc.tile_pool(name="recipp", bufs=2))
    stagp = ctx.enter_context(tc.tile_pool(name="stagp", bufs=2))
    outp = ctx.enter_context(tc.tile_pool(name="outp", bufs=3))
    dramp = ctx.enter_context(tc.tile_pool(name="dramp", bufs=3, space="DRAM"))

    ps_st = ctx.enter_context(tc.tile_pool(name="ps_st", bufs=2, space="PSUM"))
    ps_av = ctx.enter_context(tc.tile_pool(name="ps_av", bufs=2, space="PSUM"))
    ps_pj = ctx.enter_context(tc.tile_pool(name="ps_pj", bufs=2, space="PSUM"))

    # ---- prelude-critical DMAs, bandwidth-staged ----------------------------
    # Wave 1 (parallel queues): kin si0 (sync), wk (gpsimd), qin si0 (scalar).
    # Wave 2: vin g0 (sync), wv (gpsimd), wq (scalar).  Everything else (kin
    # si1-3, wo, qin si1-3, vin g1-3) is deferred so it doesn't steal HBM
    # bandwidth from the pipeline-critical prelude bytes.
    kin = {}

    def load_kin(si, eng=None):
        t = kinp.tile([P, NKC, 512], BF16, tag="kin", name=f"kin{si}")
        (eng or nc.sync).dma_start(out=t, in_=kt_r[:, :, ts(si, 512)])
        kin[si] = t

    # kin si0 in two key-slabs (256 keys each) so the first kT matmuls start
    # after only 512KB of input bytes
    kin0 = kinp.tile([P, NKC, 512], BF16, tag="kin", name="kin0")
    nc.sync.dma_start(out=kin0[:, :, 0:256], in_=kt_r[:, :, 0:256])
    nc.sync.dma_start(out=kin0[:, :, 256:512], in_=kt_r[:, :, 256:512])
    kin[0] = kin0
    # weight loads split by m-columns: the whole pc=0 pipeline (kT[0]/qT[0])
    # only touches the m=0 column slice (256KB); the rest streams later
    wk_sb = persist.tile([P, NKC, DOUT], BF16, tag="wk")
    nc.gpsimd.dma_start(out=wk_sb[:, :, 0:P], in_=wkt_r[:, :, 0:P])
    bqk_sb = persist.tile([P, 2 * NM], F32, tag="bqk")
    nc.gpsimd.dma_start(out=bqk_sb, in_=bqk_r)
    wq_sb = persist.tile([P, NKC, DOUT], BF16, tag="wq")
    nc.gpsimd.dma_start(out=wq_sb[:, :, 0:P], in_=wqt_r[:, :, 0:P])
    wv_sb = persist.tile([P, NKC, DOUT], BF16, tag="wv")
    nc.gpsimd.dma_start(out=wv_sb, in_=wvt_r)
    bv_rep = persist.tile([P, DOUT], F32, tag="bvrep")
    bv_bcast = bass.AP(
        tensor=bv_ap.tensor, offset=bv_ap.offset, ap=[[0, P]] + list(bv_ap.ap)
    )
    nc.gpsimd.dma_start(out=bv_rep, in_=bv_bcast)
    nc.gpsimd.dma_start(out=wk_sb[:, :, P:DOUT], in_=wkt_r[:, :, P:DOUT])
    nc.gpsimd.dma_start(out=wq_sb[:, :, P:DOUT], in_=wqt_r[:, :, P:DOUT])

    # ---- persistent activations (bf16) --------------------------------------
    # qT / kT: [dout, s] as 4 chunk-tiles of [128, 2048] (chunk = head pair)
    qT = [persist.tile([P, S], BF16, tag=f"qT{m}", name=f"qT{m}") for m in range(NM)]
    kT = [persist.tile([P, S], BF16, tag=f"kT{m}", name=f"kT{m}") for m in range(NM)]
    # v: [s, head, dk+1] tiles; col 64 of each head block holds ones so the
    # AV matmul's 65th output row accumulates the softmax denominator
    v_sb = [
        persist.tile([P, 8, 65], BF16, tag=f"v{i}", name=f"v{i}") for i in range(NK16)
    ]
    for i in range(NK16):
        nc.vector.memset(v_sb[i][:, :, 64:65], 1.0)
    # attn_outT: [dout, s] as 4 chunk-tiles (rows 0-63 even head, 64-127 odd)
    aoT = [persist.tile([P, S], BF16, tag=f"aoT{m}", name=f"aoT{m}") for m in range(NM)]

    # wo is only needed by the O-projection (pc=3) — DMA'd mid-attention
    wo_sb = persist.tile([P, NM, D], BF16, tag="wo")

    qin = {}

    def load_qin(si, eng=None):
        if si not in qin:
            qin[si] = qinp.tile([P, NKC, 512], BF16, tag="qin", name=f"qin{si}")
        (eng or nc.sync).dma_start(out=qin[si], in_=qt_r[:, :, ts(si, 512)])

    vin = {}

    def load_vin(g, eng=None):
        t = vinp.tile([P, NKC, 512], BF16, tag="vin", name=f"vin{g}")
        (eng or nc.sync).dma_start(out=t, in_=vt_r[:, :, ts(g, 512)])
        vin[g] = t

    # ---- exp with immediate bias/scale (skips the bias-AP read the default
    # activation path forces; falls back to the stock op if rejected) ---------
    def exp_imm(out, in_, scale):
        eng = nc.scalar
        imm = lambda v: mybir.ImmediateValue(dtype=mybir.dt.float32, value=v)
        return eng.add_instruction(
            mybir.InstActivation(
                name=eng.bass.get_next_instruction_name(),
                func=FP.Exp,
                ins=[eng.lower_ap(in_), imm(0.0), imm(scale), imm(0.0)],
                outs=[eng.lower_ap(out)],
            )
        )

    # ---- projection chunk emitters ------------------------------------------
    def proj_chunk(w_sb, xin, m, dst, si, bias_base):
        """dst[m][:, si*512:(si+1)*512] = (W[m] @ x[si]) + b[m]  (8 MMs)."""
        ps = ps_pj.tile([P, 512], F32, tag="pj", name="pspj")
        for kc in range(NKC):
            nc.tensor.matmul(
                ps,
                lhsT=w_sb[:, kc, ts(m, P)],
                rhs=xin[:, kc, :],
                start=(kc == 0),
                stop=(kc == NKC - 1),
                skip_group_check=True,
            )
        c = bias_base + m
        nc.vector.tensor_add(
            out=dst[m][:, ts(si, 512)],
            in0=ps,
            in1=bqk_sb[:, c : c + 1].to_broadcast([P, 512]),
        )

    def vproj_block(g, j):
        """v_sb[4g+j][:, :, 0:64] = (v[si16] @ Wv^T) + bv  (8 MMs)."""
        si16 = g * 4 + j
        ps = ps_pj.tile([P, 512], F32, tag="pj", name="pspjv")
        for kc in range(NKC):
            nc.tensor.matmul(
                ps,
                lhsT=vin[g][:, kc, ts(j, P)],
                rhs=wv_sb[:, kc, :],
                start=(kc == 0),
                stop=(kc == NKC - 1),
                skip_group_check=True,
            )
        nc.vector.tensor_add(
            out=v_sb[si16][:, :, 0:64],
            in0=ps.rearrange("p (h d) -> p h d", h=8),
            in1=bv_rep.rearrange("p (h d) -> p h d", h=8),
        )

    # ---- prelude: unblock the attention pipeline ASAP -----------------------
    load_qin(0, eng=nc.scalar)
    load_vin(0, eng=nc.sync)
    # kT[0] si0 in two 256-key slabs so ST(ki=0,1) unblocks after slab 0
    for sl in range(2):
        ps = ps_pj.tile([P, 512], F32, tag="pj", name="pspj")
        for kc in range(NKC):
            nc.tensor.matmul(
                ps[:, 0:256],
                lhsT=wk_sb[:, kc, 0:P],
                rhs=kin0[:, kc, sl * 256 : sl * 256 + 256],
                start=(kc == 0),
                stop=(kc == NKC - 1),
                skip_group_check=True,
            )
        nc.vector.tensor_add(
            out=kT[0][:, sl * 256 : sl * 256 + 256],
            in0=ps[:, 0:256],
            in1=bqk_sb[:, NM : NM + 1].to_broadcast([P, 256]),
        )
    proj_chunk(wq_sb, qin[0], 0, qT, 0, 0)
    for j in range(4):
        vproj_block(0, j)

    # ---- spill queue: remaining projection work, deadline-paced -------------
    # Each entry is (deadline_iter, emit_fn); iter = pc*64 + qi*16 + ki of the
    # attention iteration that first consumes the chunk (with slack baked in).
    items = []

    def sp_k(m, si, dl, load=False):
        def emit(m=m, si=si, load=load):
            if load:
                load_kin(si)
            proj_chunk(wk_sb, kin[si], m, kT, si, NM)

        items.append((dl, emit))

    def sp_q(m, si, dl, load=False):
        def emit(m=m, si=si, load=load):
            if load:
                load_qin(si)
            proj_chunk(wq_sb, qin[si], m, qT, si, 0)

        items.append((dl, emit))

    def sp_v(g, j, dl):
        def emit(g=g, j=j):
            if j == 0:
                load_vin(g)
            vproj_block(g, j)

        items.append((dl, emit))

    for si in range(1, NQ):
        sp_k(0, si, 4 * si, load=True)
    for g in range(1, NQ):
        for j in range(4):
            sp_v(g, j, 4 * g + j + 4)
    sp_q(0, 1, 16, load=True)
    sp_q(0, 2, 32, load=True)
    # qT[1..3]@si0 must be emitted before the qin si3 load reuses its slot
    sp_q(1, 0, 40)
    sp_q(2, 0, 41)
    sp_q(3, 0, 42)
    sp_q(0, 3, 48, load=True)
    for m in range(1, NM):
        for si in range(NQ):
            sp_k(m, si, 64 * m + 4 * si)
    for m in range(1, NM):
        for si in range(1, NQ):
            sp_q(m, si, 64 * m + 16 * si)
    items.sort(key=lambda e: e[0])
    spill = deque(items)

    def drain(it):
        """Deadline-driven: emit chunks ~30 iters before first use (the spill
        matmuls only get TensorE's spare ~0.35us/iter, so a chunk needs many
        iterations of headroom); spread the surplus so filler work survives
        into the late ACT-bound phase."""
        hit = False
        while spill and spill[0][0] <= it + 30:
            spill.popleft()[1]()
            hit = True
        if (
            not hit
            and spill
            and it % 3 == 2
            and len(spill) > (4 * 64 - it) // 10
        ):
            spill.popleft()[1]()

    def drain_rest():
        while spill:
            spill.popleft()[1]()

    # ---- O-projection half-chunk (4 MMs; small enough not to displace exp) --
    def oproj_half(si16, n2):
        ps = ps_pj.tile([P, 512], F32, tag="pj", name="pso")
        for c in range(NM):
            nc.tensor.matmul(
                ps,
                lhsT=aoT[c][:, ts(si16, P)],
                rhs=wo_sb[:, c, ts(n2, 512)],
                start=(c == 0),
                stop=(c == NM - 1),
                skip_group_check=True,
            )
        osb = outp.tile([P, 512], F32, tag="osb")
        nc.vector.tensor_copy(out=osb, in_=ps)
        nc.gpsimd.dma_start(out=out_r[:, si16, ts(n2, 512)], in_=osb)

    def oproj_chunk(si16):
        oproj_half(si16, 0)
        oproj_half(si16, 1)

    # ---- attention (scores kept transposed: tiles are S^T[k, q]) ------------
    # head pair pc: even head = partitions/cols 0-63, odd head = 64-127
    for pc in range(NM):
        hh = 2 * pc
        if pc == 2:
            # load the O-projection weights well before pc=3 needs them
            # (sync queue: FIFO-staged behind the earlier input loads)
            nc.sync.dma_start(out=wo_sb, in_=wot_r)
        for qi in range(NQ):
            av_e = ps_av.tile([P, 512], F32, tag="av", name="av_e")  # rows 0-64
            av_o = ps_av.tile([P, 512], F32, tag="av", name="av_o")
            for ki in range(NK16):
                st = ps_st.tile([P, 1024], F32, tag="st", name="st")
                nc.tensor.matmul(
                    st[:, 0:512],
                    lhsT=kT[pc][0:64, ts(ki, P)],
                    rhs=qT[pc][0:64, ts(qi, 512)],
                    start=True,
                    stop=True,
                )
                nc.tensor.matmul(
                    st[:, 512:1024],
                    lhsT=kT[pc][64:128, ts(ki, P)],
                    rhs=qT[pc][64:128, ts(qi, 512)],
                    start=True,
                    stop=True,
                    skip_group_check=True,
                )
                et = etp.tile([P, 1024], BF16, tag="et", name="et")
                exp_imm(out=et, in_=st, scale=0.125)
                first = ki == 0
                last = ki == NK16 - 1
                nc.tensor.matmul(
                    av_e[0:65],
                    lhsT=v_sb[ki][:, hh, :],
                    rhs=et[:, 0:512],
                    start=first,
                    stop=last,
                    skip_group_check=True,
                )
                nc.tensor.matmul(
                    av_o[0:65],
                    lhsT=v_sb[ki][:, hh + 1, :],
                    rhs=et[:, 512:1024],
                    start=first,
                    stop=last,
                    skip_group_check=True,
                )
                drain(64 * pc + 16 * qi + ki)
                # O-projection for the previous qi's s-range: six half-chunks
                # spread through the ki loop; the last chunk is reserved as a
                # filler bridging this qi's normalization latency
                if pc == NM - 1 and qi > 0 and ki % 2 == 1 and ki < 12:
                    oproj_half(4 * (qi - 1) + ki // 4, (ki // 2) % 2)
            # free the PSUM banks fast, then normalize from SBUF
            ae = avsb.tile([P, 512], F32, tag="ae", name="ae")
            ao = avsb.tile([P, 512], F32, tag="ae", name="ao")
            nc.vector.tensor_copy(out=ae[0:65], in_=av_e[0:65])
            nc.vector.tensor_copy(out=ao[0:65], in_=av_o[0:65])
            # denominators: bounce [2,512] to DRAM, reshape to [128,8] so the
            # iterative-divide reciprocal runs across partitions, not 1 lane
            scr = dramp.tile([2, 512], F32, tag="scr", name="scr")
            nc.sync.dma_start(out=scr[0:1, :], in_=ae[64:65, :])
            nc.sync.dma_start(out=scr[1:2, :], in_=ao[64:65, :])
            den = recipp.tile([P, 8], F32, tag="den", name="den")
            scr_flat = scr[:, :].rearrange("a b -> (a b)").rearrange(
                "(p f) -> p f", p=P
            )
            nc.sync.dma_start(out=den, in_=scr_flat)
            denr = recipp.tile([P, 8], F32, tag="denr", name="denr")
            nc.vector.reciprocal(out=denr, in_=den)
            scr2 = dramp.tile([P, 8], F32, tag="scr2", name="scr2")
            nc.sync.dma_start(out=scr2, in_=denr)
            rep = recipp.tile([P, 1024], F32, tag="rep", name="rep")
            s2f = scr2[:, :].rearrange("p f -> (p f)")
            nc.sync.dma_start(
                out=rep[0:64, :],
                in_=bass.AP(
                    tensor=s2f.tensor, offset=s2f.offset, ap=[[0, 64]] + list(s2f.ap)
                ),
            )
            last_block = pc == NM - 1 and qi == NQ - 1
            if not last_block:
                nc.vector.tensor_mul(
                    out=aoT[pc][0:64, ts(qi, 512)], in0=ae[0:64], in1=rep[0:64, 0:512]
                )
                stag = stagp.tile([P, 512], BF16, tag="stag", name="stag")
                nc.vector.tensor_mul(
                    out=stag[0:64, :], in0=ao[0:64], in1=rep[0:64, 512:1024]
                )
                nc.sync.dma_start(
                    out=aoT[pc][64:128, ts(qi, 512)], in_=stag[0:64, :]
                )
                if pc == NM - 1 and qi > 0:
                    # reserved filler: keep TensorE warm behind the norm chain
                    oproj_chunk(4 * (qi - 1) + 3)
            else:
                # final block: normalize in 128-col pieces, each immediately
                # feeding its O-projection chunk so the tail pipelines;
                # qi=2's last chunk bridges the reciprocal chain latency
                oproj_chunk(4 * (qi - 1) + 3)
                for piece in range(4):
                    cs = qi * 512 + piece * 128
                    ps_lo = slice(piece * 128, piece * 128 + 128)
                    ps_hi = slice(512 + piece * 128, 512 + piece * 128 + 128)
                    nc.vector.tensor_mul(
                        out=aoT[pc][0:64, cs : cs + 128],
                        in0=ae[0:64, piece * 128 : piece * 128 + 128],
                        in1=rep[0:64, ps_lo],
                    )
                    stag = stagp.tile([P, 128], BF16, tag="stagf", name="stagf")
                    nc.vector.tensor_mul(
                        out=stag[0:64, :],
                        in0=ao[0:64, piece * 128 : piece * 128 + 128],
                        in1=rep[0:64, ps_hi],
                    )
                    nc.sync.dma_start(
                        out=aoT[pc][64:128, cs : cs + 128], in_=stag[0:64, :]
                    )
                    oproj_chunk(4 * qi + piece)

    drain_rest()


def _build():
    global _cached_nc
    if _cached_nc is not None:
        return _cached_nc
    nc = bacc.Bacc("TRN2", target_bir_lowering=False, debug=False)
    io = {
        "qt": nc.dram_tensor("qt", [D, S], BF16, kind="ExternalInput"),
        "kt": nc.dram_tensor("kt", [D, S], BF16, kind="ExternalInput"),
        "vt": nc.dram_tensor("vt", [D, S], BF16, kind="ExternalInput"),
        "wqt": nc.dram_tensor("wqt", [D, DOUT], BF16, kind="ExternalInput"),
        "wkt": nc.dram_tensor("wkt", [D, DOUT], BF16, kind="ExternalInput"),
        "wvt": nc.dram_tensor("wvt", [D, DOUT], BF16, kind="ExternalInput"),
        "wot": nc.dram_tensor("wot", [DOUT, D], BF16, kind="ExternalInput"),
        "bqk": nc.dram_tensor("bqk", [2 * DOUT], F32, kind="ExternalInput"),
        "bv": nc.dram_tensor("bv", [DOUT], F32, kind="ExternalInput"),
        "out": nc.dram_tensor("out", [S, D], F32, kind="ExternalOutput"),
    }
    with tile.TileContext(nc) as tc:
        with ExitStack() as ctx:
            _emit(ctx, tc, io)
    nc.compile()
    _cached_nc = nc
    return nc


def make_in_maps(Q, K, V, Wq, bq, Wk, bk, Wv, bv, Wo):
    bf = lambda a: np.ascontiguousarray(np.asarray(a, np.float32)).astype(
        ml_dtypes.bfloat16
    )
    f = lambda a: np.ascontiguousarray(a, dtype=np.float32)
    in_maps = []
    for c in range(N_CORES):
        b = c // 2
        lo = (c % 2) * DOUT
        sl = slice(lo, lo + DOUT)
        in_maps.append(
            {
                "qt": bf(np.asarray(Q, np.float32)[b].T),
                "kt": bf(np.asarray(K, np.float32)[b].T),
                "vt": bf(np.asarray(V, np.float32)[b].T),
                "wqt": bf(np.asarray(Wq, np.float32)[sl, :].T),
                "wkt": bf(np.asarray(Wk, np.float32)[sl, :].T),
                "wvt": bf(np.asarray(Wv, np.float32)[sl, :].T),
                "wot": bf(np.asarray(Wo, np.float32)[:, sl].T),
                # partition-major shuffle: bqk[p, c] = bias[c*128 + p]
                "bqk": f(
                    np.concatenate(
                        [
                            np.asarray(bq[sl]).reshape(NM, P).T,
                            np.asarray(bk[sl]).reshape(NM, P).T,
                        ],
                        axis=1,
                    ).reshape(-1)
                ),
                "bv": f(bv[sl]),
            }
        )
    return in_maps


def gather_output(results, bo):
    out = np.empty((B, S, D), dtype=np.float32)
    bo = np.asarray(bo, dtype=np.float32)
    for b in range(B):
        out[b] = results[2 * b]["out"] + results[2 * b + 1]["out"] + bo
    return out


def _numpy_fallback(Q, K, V, mask, Wq, bq, Wk, bk, Wv, bv, Wo, bo):
    """Exact reference math in numpy (only used if mask isn't all-ones)."""
    H, dk = 16, 64
    out = np.empty((B, S, D), dtype=np.float32)
    for b in range(B):
        q = (Q[b] @ Wq.T + bq).reshape(S, H, dk).transpose(1, 0, 2)
        k = (K[b] @ Wk.T + bk).reshape(S, H, dk).transpose(1, 0, 2)
        v = (V[b] @ Wv.T + bv).reshape(S, H, dk).transpose(1, 0, 2)
        o = np.empty((H, S, dk), dtype=np.float32)
        for h in range(H):
            s = (q[h] @ k[h].T) / np.sqrt(np.float32(dk))
            s = np.where(mask[b] == 0, np.float32(-1.0e9), s)
            s = s - s.max(axis=-1, keepdims=True)
            e = np.exp(s)
            a = e / e.sum(axis=-1, keepdims=True)
            o[h] = a @ v[h]
        out[b] = o.transpose(1, 0, 2).reshape(S, H * dk) @ Wo.T + bo
    return out


def kernel(Q, K, V, mask, Wq, bq, Wk, bk, Wv, bv, Wo, bo):
    Q = np.asarray(Q, dtype=np.float32)
    K = np.asarray(K, dtype=np.float32)
    V = np.asarray(V, dtype=np.float32)
    Wq = np.asarray(Wq, dtype=np.float32)
    Wk = np.asarray(Wk, dtype=np.float32)
    Wv = np.asarray(Wv, dtype=np.float32)
    Wo = np.asarray(Wo, dtype=np.float32)
    bq = np.asarray(bq, dtype=np.float32)
    bk = np.asarray(bk, dtype=np.float32)
    bv = np.asarray(bv, dtype=np.float32)
    bo = np.asarray(bo, dtype=np.float32)
    mask_np = np.asarray(mask)

    if not np.all(mask_np != 0):
        return _numpy_fallback(Q, K, V, mask_np, Wq, bq, Wk, bk, Wv, bv, Wo, bo)

    nc = _build()
    in_maps = make_in_maps(Q, K, V, Wq, bq, Wk, bk, Wv, bv, Wo)
    res = run_bass_kernel_spmd(nc, in_maps, list(range(N_CORES))).results
    return gather_output(res, bo)
